# revision 21
# baseline (speedup 1.0000x reference)
"""Trainium2 Bass kernel for LoopRelationalGraphConvolution.

Math (matches the jax reference):
    out[n] = relu( SCALE * sum_s  W[rel[n,s]] @ emb[neighbors[n,s]] )
    SCALE  = 1000 / (R1 * S)      (folds the mean over S and the /R1 * 1000)

Design (8 NeuronCores, data-parallel over the 8192-node batch):
  Each core owns 1024 nodes, split into 9 node-tiles ([114]*8 + [112] nodes)
  chosen by a host-side balancer so that every (tile, relation) bucket has
  <=128 edges.  All big matmuls run in fp8 DoubleRow mode (K=256 in one
  pass, 0.5 cycles/out-row) with a two-level fp8 decomposition for full
  accuracy:  x ~= xh + xl,  W ~= Wh + Wl  (each fp8_e4m3, lo = residual),
  Y = Wh@xh + Wh@xl + Wl@xh  (dropping the ~0.1% Wl@xl term).

  Per tile the device kernel:
    1. dma_gather(transpose=True) fetches the tile's 33*128 edge-slot
       embeddings from an interleaved hi/lo fp8 table ehl[v, 2d+b] so the
       16-bit transpose granularity lands (hi[d], lo[d]) byte pairs at
       partition d%128, k-tile d//128:  ET[p, c, i, b].
    2. stage-1: per relation r three DoubleRow matmuls accumulate
       Y[slot, o] into a 4-relation-wide PSUM group [128, 1024].
    3. the PSUM group is evacuated to bf16 SBUF in one wide copy
       (alternating DVE / Act engines to stay off the PE critical path).
    4. stage-2: 0/1 selection matrix (fp8) reduces edge slots into node
       rows:  out_psum[node, o] += SEL_r^T @ Y_bf16  (accumulated over r).
    5. relu(x * SCALE/A) on PSUM->SBUF evacuation, bf16 rows DMAd out.
  The device program is fully static and identical across cores (SPMD); all
  data-dependence lives in the index / selection arrays.  Host post-step
  inverse-permutes rows back to the original node order.
"""

import numpy as np
import ml_dtypes

bf16 = ml_dtypes.bfloat16
fp8 = ml_dtypes.float8_e4m3

# Problem constants (hardcoded per contract).
V = 100000
D = 256
R1 = 33          # relations incl. self-loop
N = 8192
S = 32
NCORES = 8
NPC = N // NCORES          # 1024 nodes per core
NTILES = 9                 # node-tiles per core
CAPS = [114] * 8 + [112]   # nodes per tile (uniform across cores)
ROW_BASE = np.concatenate([[0], np.cumsum(CAPS)]).tolist()
P = 128
NSLOT = R1 * P             # 4224 edge slots per tile
IDXW = NSLOT // 16         # 264 int16 idx columns (16-partition wrap)
UMAX = 32768               # compacted per-core embedding rows (int16 limit)
SCALE = 1000.0 / (R1 * S)
WA = 16.0                  # fp8 pre-scale on W; folded out in the final relu

# relation groups: stage-1 accumulates a group into one [128, GW*256] PSUM
# region, evacuated in a single wide copy.
GROUPS = [(g, min(g + 4, R1)) for g in range(0, R1, 4)]   # 8x4 + 1x1
NG = len(GROUPS)
SKEWG = 3   # stage-2 lags stage-1 by this many groups
# per-step PSUM->SBUF copy engine: 0=DVE, 1=Act, 2=Pool
COPY_PATTERN = [0, 1]
PF = 2      # tile prefetch depth
# gathers are segmented so no single transfer hogs the (exclusive) DMA
# engines; tile-0's first segment is extra small so the PE starts early.
T0_SEGS = [(0, 4), (4, 12), (12, 22), (22, R1)]   # relation ranges
T_SEGS = [(0, 11), (11, 22), (22, R1)]


# ---------------------------------------------------------------------------
# Host-side preparation
# ---------------------------------------------------------------------------

def _balance_tiles(hist):
    """Assign NPC nodes to NTILES tiles (exactly CAPS[t] nodes each),
    minimizing the max per-(tile, relation) edge count. hist: [NPC, R1].
    Greedy: hardest nodes first, place on the tile minimizing the resulting
    peak bucket."""
    order = np.argsort(-hist.max(axis=1), kind="stable")
    loads = np.zeros((NTILES, R1), dtype=np.int64)
    counts = np.zeros(NTILES, dtype=np.int64)
    tiles = [[] for _ in range(NTILES)]
    for n in order:
        h = hist[n]
        best_t, best_key = -1, None
        for t in range(NTILES):
            if counts[t] >= CAPS[t]:
                continue
            new = loads[t] + h
            key = (int(new.max()), int(loads[t].max()), int(new.sum()))
            if best_key is None or key < best_key:
                best_key, best_t = key, t
        tiles[best_t].append(int(n))
        loads[best_t] += h
        counts[best_t] += 1
    return tiles, loads


def _split2(x):
    """Two-level fp8 decomposition: x ~= hi + lo, both fp8_e4m3."""
    x = np.asarray(x, dtype=np.float32)
    hi = x.astype(fp8)
    lo = (x - hi.astype(np.float32)).astype(fp8)
    return hi, lo


def prep(emb_table, weights, neighbors, relations):
    """Build per-core device arrays. Returns (in_maps, perms)."""
    emb32 = np.asarray(emb_table, dtype=np.float32)
    wa = np.asarray(weights, dtype=np.float32).transpose(0, 2, 1) * WA  # [r,d,o]
    wh, wl = _split2(wa)
    # W_sb[p, (((r*2 + lvl)*2 + kt)*D + o)] = W{lvl}[r, kt*128+p, o]
    wsb = np.stack([wh, wl], axis=1).reshape(R1, 2, 2, 128, D)  # [r,lvl,kt,p,o]
    W_sb = np.ascontiguousarray(wsb.transpose(3, 0, 1, 2, 4)).reshape(
        128, R1 * 2 * 2 * D)

    neighbors = np.asarray(neighbors).astype(np.int64)
    relations = np.asarray(relations).astype(np.int64)

    in_maps, perms = [], []
    for c in range(NCORES):
        nb = neighbors[c * NPC:(c + 1) * NPC]                 # [NPC, S]
        rel = relations[c * NPC:(c + 1) * NPC]
        uniq, inv = np.unique(nb.ravel(), return_inverse=True)
        inv = inv.reshape(nb.shape).astype(np.int64)
        U = len(uniq)
        assert U <= UMAX, U
        hi, lo = _split2(emb32[uniq])                         # [U, D] each
        ehl = np.zeros((UMAX, 2 * D), dtype=fp8)
        ehl[:U, 0::2] = hi
        ehl[:U, 1::2] = lo

        hist = np.zeros((NPC, R1), dtype=np.int64)
        np.add.at(hist, (np.repeat(np.arange(NPC), S), rel.ravel()), 1)
        tiles, loads = _balance_tiles(hist)
        assert loads.max() <= P, f"balance failed: max bucket {loads.max()}"

        idx_all = np.zeros((NTILES, 128, IDXW), dtype=np.int16)
        sel_all = np.zeros((NTILES, 128, NSLOT), dtype=fp8)
        perm = []
        for t, nodes in enumerate(tiles):
            nodes = np.array(nodes, dtype=np.int64)
            ncnt = len(nodes)
            assert ncnt == CAPS[t]
            perm.extend((c * NPC + nodes).tolist())
            # edges of this tile
            er = rel[nodes].ravel()                            # relation per edge
            ei = inv[nodes].ravel()                            # compact nbr id
            ej = np.repeat(np.arange(ncnt), S)                 # local node idx
            order = np.argsort(er, kind="stable")
            er_s, ei_s, ej_s = er[order], ei[order], ej[order]
            # position within relation group
            start = np.searchsorted(er_s, np.arange(R1))
            pos = np.arange(ncnt * S) - start[er_s]
            slot = er_s * P + pos                              # [ncnt*S]
            slots_idx = np.zeros(NSLOT, dtype=np.int16)
            slots_idx[slot] = ei_s
            sel = np.zeros((NSLOT, 128), dtype=fp8)
            sel[slot, ej_s] = 1.0
            # idx wrap: idx i at partition i%16, col i//16 (replicated x8)
            wrapped = slots_idx.reshape(IDXW, 16).T            # [16, IDXW]
            idx_all[t] = np.tile(wrapped, (8, 1))
            # device SEL layout: [part p = slot-in-chunk, free = r*128 + node]
            sel_all[t] = np.ascontiguousarray(
                sel.reshape(R1, P, 128).transpose(1, 0, 2).reshape(P, NSLOT))
        in_maps.append({
            "emb": ehl,
            "wsb": W_sb,
            "idx": np.ascontiguousarray(idx_all.reshape(NTILES * 128, IDXW)),
            "sel": np.ascontiguousarray(sel_all.reshape(NTILES * 128, NSLOT)),
        })
        perms.append(np.array(perm, dtype=np.int64))

    return in_maps, perms


# ---------------------------------------------------------------------------
# Numpy emulation (fp8/bf16-faithful) for validation
# ---------------------------------------------------------------------------

def emulate_core(in_map):
    ehl = in_map["emb"]                                        # [UMAX, 2D] fp8
    xh = ehl[:, 0::2].astype(np.float32)
    xl = ehl[:, 1::2].astype(np.float32)
    wsb = in_map["wsb"].reshape(128, R1, 2, 2, D)              # [p,r,lvl,kt,o]
    w = np.ascontiguousarray(wsb.transpose(1, 2, 3, 0, 4)).reshape(
        R1, 2, 2 * 128, D).astype(np.float32)                  # [r, lvl, d, o]
    idx = in_map["idx"].reshape(NTILES, 128, IDXW)
    sel = in_map["sel"].reshape(NTILES, 128, NSLOT)
    out = np.zeros((NPC, D), dtype=np.float32)
    for t in range(NTILES):
        slots_idx = idx[t, :16, :].T.reshape(NSLOT)            # unwrap
        Xh = xh[slots_idx]                                     # [NSLOT, D]
        Xl = xl[slots_idx]
        out_acc = np.zeros((128, D), dtype=np.float32)
        for r in range(R1):
            sl = slice(r * P, (r + 1) * P)
            Y = (Xh[sl] @ w[r, 0] + Xl[sl] @ w[r, 0] + Xh[sl] @ w[r, 1])
            Yb = Y.astype(bf16).astype(np.float32)             # PSUM->SBUF bf16
            selr = sel[t][:, r * 128:(r + 1) * 128].astype(np.float32)
            out_acc += selr.T @ Yb
        base, ncnt = ROW_BASE[t], CAPS[t]
        res = np.maximum(out_acc[:ncnt] * (SCALE / WA), 0.0)
        out[base:base + ncnt] = res.astype(bf16).astype(np.float32)
    return out


def emulate(emb_table, weights, neighbors, relations):
    in_maps, perms = prep(emb_table, weights, neighbors, relations)
    full = np.zeros((N, D), dtype=np.float32)
    for c in range(NCORES):
        full[perms[c]] = emulate_core(in_maps[c])
    return full


# ---------------------------------------------------------------------------
# Bass program
# ---------------------------------------------------------------------------

def build_program():
    import concourse.bacc as bacc
    import concourse.tile as tile
    import concourse.mybir as mybir

    nc = bacc.Bacc(
        "TRN2", target_bir_lowering=False, debug=False,
        num_devices=NCORES,
    )
    BF = mybir.dt.bfloat16
    F32 = mybir.dt.float32
    I16 = mybir.dt.int16
    F8 = mybir.dt.float8e4
    DR = mybir.MatmulPerfMode.DoubleRow

    emb = nc.dram_tensor("emb", [UMAX, 2 * D], F8, kind="ExternalInput").ap()
    wsb = nc.dram_tensor("wsb", [128, R1 * 2 * 2 * D], F8,
                         kind="ExternalInput").ap()
    idx = nc.dram_tensor("idx", [NTILES * 128, IDXW], I16,
                         kind="ExternalInput").ap()
    sel = nc.dram_tensor("sel", [NTILES * 128, NSLOT], F8,
                         kind="ExternalInput").ap()
    out = nc.dram_tensor("out", [NPC, D], BF, kind="ExternalOutput").ap()

    Relu = mybir.ActivationFunctionType.Relu

    with tile.TileContext(nc) as tc:
        with (
            tc.tile_pool(name="wpool", bufs=1) as wpool,
            tc.tile_pool(name="etpool", bufs=14) as etpool,
            tc.tile_pool(name="selpool", bufs=PF + 2) as selpool,
            tc.tile_pool(name="idxpool", bufs=PF + 2) as idxpool,
            tc.tile_pool(name="ypool", bufs=SKEWG + 3) as ypool,
            tc.tile_pool(name="opool", bufs=2) as opool,
            tc.tile_pool(name="psy", bufs=3, space="PSUM") as psy,
            tc.tile_pool(name="pso", bufs=2, space="PSUM") as pso,
        ):
            # one W tile per relation-group so stage-1 of group g only
            # depends on its own chunk's DMA (tile-granular dep tracking).
            wts = {}

            def load_w(g):
                r0, r1 = GROUPS[g]
                wtg = wpool.tile([128, r1 - r0, 2, 2, D], F8,
                                 name=f"wt{g}", uniquify=False)
                nc.sync.dma_start(out=wtg[:],
                                  in_=wsb[:, r0 * 4 * D:r1 * 4 * D])
                wts[g] = wtg

            ets, sels = {}, {}

            def pre_gather(t, segs):
                idx_t = idxpool.tile([128, IDXW], I16, name="idx_t")
                nc.sync.dma_start(
                    out=idx_t[:], in_=idx[t * 128:(t + 1) * 128, :])
                parts = []
                for (ra, rb) in segs:
                    n = (rb - ra) * P
                    eth = etpool.tile([128, 4, n], F8, name="eth")
                    nc.gpsimd.dma_gather(
                        out_ap=eth[:],
                        in_ap=emb,
                        idxs_ap=idx_t[:, ra * 8:rb * 8],
                        num_idxs=n,
                        num_idxs_reg=n,
                        elem_size=2 * D,
                        transpose=True,
                        single_packet=False,
                    )
                    # true byte layout: [p][ktile c:2][slot i:n][hi/lo b:2]
                    ethv = eth[:].rearrange("p f n -> p (f n)").rearrange(
                        "p (c i b) -> p c i b", c=2, i=n, b=2)
                    parts.append((ra, rb, ethv))
                ets[t] = parts

            def pre_sel(t):
                sel_t = selpool.tile([128, NSLOT], F8, name="sel_t")
                nc.sync.dma_start(
                    out=sel_t[:], in_=sel[t * 128:(t + 1) * 128, :])
                sels[t] = sel_t

            prefetched = set()

            def prefetch(t):
                if t >= NTILES or t in prefetched:
                    return
                prefetched.add(t)
                pre_gather(t, T_SEGS)
                pre_sel(t)

            def lhs(t, r, b):
                for (ra, rb, eth) in ets[t]:
                    if ra <= r < rb:
                        o = (r - ra) * P
                        return eth[:, :, o:o + P, b:b + 1]
                raise AssertionError

            # PE warm-up: dummy matmuls over a never-written scratch tile keep
            # the PE continuously busy from t~0.6us so the p-state ramp
            # completes during the DMA pipeline fill (results discarded).
            dumt = wpool.tile([128, 512], BF, name="dumt")
            nc.vector.memset(dumt[:], 0)
            dump = psy.tile([128, 4 * D], F32, name="yp", uniquify=False)
            for i in range(24):
                nc.tensor.matmul(out=dump[:, :256], lhsT=dumt[:, :128],
                                 rhs=dumt[:, 256:512], start=True, stop=True,
                                 skip_group_check=True)

            # startup orchestration: tile-0 gathers first; only the W chunks
            # needed immediately go ahead of them in the DMA queue, the rest
            # interleave with tile prefetches so gathers aren't starved.
            pre_gather(0, T0_SEGS)
            load_w(0)
            load_w(1)
            load_w(2)
            pre_sel(0)
            load_w(3)
            prefetch(1)
            load_w(4)
            load_w(5)
            prefetch(2)
            load_w(6)
            load_w(7)
            prefetch(3)
            load_w(8)

            steps = [(t, g) for t in range(NTILES) for g in range(NG)]
            ysbs = {}
            outps = {}

            def stage2(sj):
                tj, gj = steps[sj]
                r0, r1 = GROUPS[gj]
                ysb = ysbs.pop(sj)
                outp = outps[tj]
                for r in range(r0, r1):
                    nc.tensor.matmul(
                        out=outp[:],
                        lhsT=sels[tj][:, r * P:(r + 1) * P],
                        rhs=ysb[:, (r - r0) * D:(r - r0 + 1) * D],
                        start=(r == 0), stop=(r == R1 - 1),
                    )
                if gj == NG - 1:
                    osb = opool.tile([128, D], BF, name="osb")
                    # relu(x * SCALE/WA); DVE (tensor_scalar) and Act
                    # (activation) alternate per tile to spread the load
                    if tj % 2 == 0:
                        nc.vector.tensor_scalar(
                            out=osb[:], in0=outp[:], scalar1=SCALE / WA,
                            scalar2=0.0, op0=mybir.AluOpType.mult,
                            op1=mybir.AluOpType.max)
                    else:
                        nc.scalar.activation(out=osb[:], in_=outp[:],
                                             func=Relu, scale=SCALE / WA)
                    base, ncnt = ROW_BASE[tj], CAPS[tj]
                    nc.sync.dma_start(
                        out=out[base:base + ncnt, :], in_=osb[:ncnt, :])
                    del outps[tj]
                    del sels[tj], ets[tj]

            for si, (t, g) in enumerate(steps):
                if g == 0:
                    outps[t] = pso.tile([128, D], F32, name="outp")
                    prefetch(t + PF + 1)
                r0, r1 = GROUPS[g]
                gw = r1 - r0
                yp = psy.tile([128, 4 * D], F32, name="yp")
                wtg = wts[g]
                for r in range(r0, r1):
                    o0 = (r - r0) * D
                    ydst = yp[:, o0:o0 + D]
                    wh = wtg[:, r - r0, 0]
                    wl = wtg[:, r - r0, 1]
                    nc.tensor.matmul(out=ydst, lhsT=lhs(t, r, 0), rhs=wh,
                                     start=True, stop=False, perf_mode=DR)
                    nc.tensor.matmul(out=ydst, lhsT=lhs(t, r, 1), rhs=wh,
                                     start=False, stop=False, perf_mode=DR)
                    nc.tensor.matmul(out=ydst, lhsT=lhs(t, r, 0), rhs=wl,
                                     start=False, stop=True, perf_mode=DR)
                ysb = ypool.tile([128, 4 * D], BF, name="ysb")
                eng = COPY_PATTERN[si % len(COPY_PATTERN)]
                if eng == 0:
                    nc.vector.tensor_copy(out=ysb[:, :gw * D],
                                          in_=yp[:, :gw * D])
                elif eng == 1:
                    nc.scalar.copy(out=ysb[:, :gw * D], in_=yp[:, :gw * D])
                else:
                    nc.gpsimd.tensor_copy(out=ysb[:, :gw * D],
                                          in_=yp[:, :gw * D])
                ysbs[si] = ysb
                if si - SKEWG >= 0:
                    stage2(si - SKEWG)
            for sj in range(len(steps) - SKEWG, len(steps)):
                stage2(sj)

    nc.compile()
    return nc


_NC_CACHE = []


def _get_program():
    if not _NC_CACHE:
        _NC_CACHE.append(build_program())
    return _NC_CACHE[0]


# ---------------------------------------------------------------------------
# Entry point
# ---------------------------------------------------------------------------

def kernel(emb_table, weights, neighbors, relations):
    from concourse import bass_utils

    in_maps, perms = prep(emb_table, weights, neighbors, relations)
    nc = _get_program()
    res = bass_utils.run_bass_kernel_spmd(
        nc, in_maps, core_ids=list(range(NCORES)),
    )
    full = np.zeros((N, D), dtype=np.float32)
    for c in range(NCORES):
        full[perms[c]] = res.results[c]["out"].astype(np.float32)
    return full


# revision 28
# speedup vs baseline: 1.0335x; 1.0335x over previous
"""Trainium2 Bass kernel for LoopRelationalGraphConvolution.

Math (matches the jax reference):
    out[n] = relu( SCALE * sum_s  W[rel[n,s]] @ emb[neighbors[n,s]] )
    SCALE  = 1000 / (R1 * S)      (folds the mean over S and the /R1 * 1000)

Design (8 NeuronCores, data-parallel over the 8192-node batch):
  Each core owns 1024 nodes, split into 9 node-tiles ([114]*8 + [112] nodes)
  chosen by a host-side balancer so that every (tile, relation) bucket has
  <=128 edges.  Per tile the device kernel:
    1. dma_gather(transpose=True): fetches the tile's 33*128 edge-slot
       embeddings (bf16, compacted per-core int16 ids) with the embedding dim
       landing on SBUF partitions:  ET[p, c, i] = emb[idx_i][c*128+p].
    2. stage-1 matmuls: per relation-chunk r (128 slots), in 2 K-chunks over D:
       Y[slot, o] += ET[:, c, slots]^T @ W_r[c]       (PSUM, f32)
    3. stage-2 matmul: 0/1 selection matrix reduces edge slots into node rows:
       out_psum[node, o] += SEL_r^T @ Y_bf16          (accumulated over all r)
    4. relu on PSUM->SBUF evacuation, DMA node rows to DRAM.
  The device program is fully static and identical across cores (SPMD); all
  data-dependence lives in the index / selection arrays.  Host post-step
  inverse-permutes rows back to the original node order.
"""

import numpy as np
import ml_dtypes

bf16 = ml_dtypes.bfloat16
fp8 = ml_dtypes.float8_e4m3

# Problem constants (hardcoded per contract).
V = 100000
D = 256
R1 = 33          # relations incl. self-loop
N = 8192
S = 32
NCORES = 8
NPC = N // NCORES          # 1024 nodes per core
NTILES = 9                 # node-tiles per core
CAPS = [114] * 8 + [112]   # nodes per tile (uniform across cores)
ROW_BASE = np.concatenate([[0], np.cumsum(CAPS)]).tolist()
P = 128
NSLOT = R1 * P             # 4224 edge slots per tile
GSPLIT = [0, 2, 6, 12, 19, 26, R1]   # gather segment chunk boundaries
GSEG = [(a * P, b * P) for a, b in zip(GSPLIT, GSPLIT[1:])]
IDXW = NSLOT // 16         # 264 int16 idx columns (16-partition wrap)
UMAX = 32768               # compacted per-core embedding rows (int16 limit)
SCALE = 1000.0 / (R1 * S)

# Software-pipeline skew between stage-1 and stage-2 of consecutive chunks,
# so the PE never stalls on the PSUM->SBUF copy of the current chunk.
SKEW = 6
PF = 3      # tile prefetch depth


# ---------------------------------------------------------------------------
# Host-side preparation
# ---------------------------------------------------------------------------

def _balance_tiles(hist):
    """Assign NPC nodes to NTILES tiles (exactly CAPS[t] nodes each),
    minimizing the max per-(tile, relation) edge count. hist: [NPC, R1].
    Greedy: hardest nodes first, place on the tile minimizing the resulting
    peak bucket."""
    order = np.argsort(-hist.max(axis=1), kind="stable")
    loads = np.zeros((NTILES, R1), dtype=np.int64)
    counts = np.zeros(NTILES, dtype=np.int64)
    tiles = [[] for _ in range(NTILES)]
    for n in order:
        h = hist[n]
        best_t, best_key = -1, None
        for t in range(NTILES):
            if counts[t] >= CAPS[t]:
                continue
            new = loads[t] + h
            key = (int(new.max()), int(loads[t].max()), int(new.sum()))
            if best_key is None or key < best_key:
                best_key, best_t = key, t
        tiles[best_t].append(int(n))
        loads[best_t] += h
        counts[best_t] += 1
    return tiles, loads


def prep(emb_table, weights, neighbors, relations):
    """Build per-core device arrays. Returns (in_maps, perms)."""
    emb_bf = np.asarray(emb_table).astype(bf16)
    w = np.asarray(weights, dtype=np.float32) * SCALE         # [R1, D_out, D_in]
    # W_sb[p, (r*2 + c)*D + o] = w[r, o, c*128+p]
    w_rdo = np.ascontiguousarray(w.transpose(0, 2, 1))        # [r, d, o]
    W_sb = np.ascontiguousarray(
        w_rdo.reshape(R1, 2, 128, D).transpose(2, 0, 1, 3)    # [p, r, c, o]
    ).reshape(128, R1 * 2 * D).astype(bf16)

    neighbors = np.asarray(neighbors).astype(np.int64)
    relations = np.asarray(relations).astype(np.int64)

    in_maps, perms = [], []
    for c in range(NCORES):
        nb = neighbors[c * NPC:(c + 1) * NPC]                 # [NPC, S]
        rel = relations[c * NPC:(c + 1) * NPC]
        uniq, inv = np.unique(nb.ravel(), return_inverse=True)
        inv = inv.reshape(nb.shape).astype(np.int64)
        U = len(uniq)
        assert U <= UMAX, U
        emb_c = np.zeros((UMAX, D), dtype=bf16)
        emb_c[:U] = emb_bf[uniq]

        hist = np.zeros((NPC, R1), dtype=np.int64)
        np.add.at(hist, (np.repeat(np.arange(NPC), S), rel.ravel()), 1)
        tiles, loads = _balance_tiles(hist)
        assert loads.max() <= P, f"balance failed: max bucket {loads.max()}"

        idx_all = np.zeros((NTILES, 128, IDXW), dtype=np.int16)
        sel_all = np.zeros((NTILES, 128, NSLOT), dtype=fp8)
        perm = []
        for t, nodes in enumerate(tiles):
            nodes = np.array(nodes, dtype=np.int64)
            ncnt = len(nodes)
            assert ncnt == CAPS[t]
            perm.extend((c * NPC + nodes).tolist())
            # edges of this tile
            er = rel[nodes].ravel()                            # relation per edge
            ei = inv[nodes].ravel()                            # compact nbr id
            ej = np.repeat(np.arange(ncnt), S)                 # local node idx
            order = np.argsort(er, kind="stable")
            er_s, ei_s, ej_s = er[order], ei[order], ej[order]
            # position within relation group
            start = np.searchsorted(er_s, np.arange(R1))
            pos = np.arange(ncnt * S) - start[er_s]
            slot = er_s * P + pos                              # [ncnt*S]
            slots_idx = np.zeros(NSLOT, dtype=np.int16)
            slots_idx[slot] = ei_s
            sel = np.zeros((NSLOT, 128), dtype=fp8)
            sel[slot, ej_s] = 1.0
            # idx wrap per gather segment: idx i at partition i%16, col i//16
            wrapped = np.concatenate(
                [slots_idx[a:b].reshape((b - a) // 16, 16).T
                 for a, b in GSEG], axis=1)                    # [16, IDXW]
            idx_all[t] = np.tile(wrapped, (8, 1))
            # device SEL layout: [part p = slot-in-chunk, free = r*128 + node]
            sel_all[t] = np.ascontiguousarray(
                sel.reshape(R1, P, 128).transpose(1, 0, 2).reshape(P, NSLOT))
        in_maps.append({
            "emb": emb_c,
            "wsb": W_sb,
            "idx": np.ascontiguousarray(idx_all.reshape(NTILES * 128, IDXW)),
            "sel": np.ascontiguousarray(sel_all.reshape(NTILES * 128, NSLOT)),
        })
        perms.append(np.array(perm, dtype=np.int64))

    return in_maps, perms


# ---------------------------------------------------------------------------
# Numpy emulation (bf16-faithful) for validation
# ---------------------------------------------------------------------------

def emulate_core(in_map):
    emb = in_map["emb"]                                        # [UMAX, D] bf16
    wsb = in_map["wsb"].reshape(128, R1, 2, D)                 # [p, r, c, o]
    idx = in_map["idx"].reshape(NTILES, 128, IDXW)
    sel = in_map["sel"].reshape(NTILES, 128, NSLOT)
    out = np.zeros((NPC, D), dtype=np.float32)
    for t in range(NTILES):
        parts, col = [], 0
        for a, b in GSEG:
            w = (b - a) // 16
            parts.append(idx[t, :16, col:col + w].T.reshape(b - a))
            col += w
        slots_idx = np.concatenate(parts)                      # unwrap
        X = emb[slots_idx]                                     # [NSLOT, D] bf16
        out_acc = np.zeros((128, D), dtype=np.float32)
        for r in range(R1):
            Xr = X[r * P:(r + 1) * P].astype(np.float32)       # [128, D]
            Y = (Xr[:, :128] @ wsb[:, r, 0, :].astype(np.float32)
                 + Xr[:, 128:] @ wsb[:, r, 1, :].astype(np.float32))
            Yb = Y.astype(bf16).astype(np.float32)             # PSUM->SBUF bf16
            selr = sel[t][:, r * 128:(r + 1) * 128].astype(np.float32)
            out_acc += selr.T @ Yb
        base, ncnt = ROW_BASE[t], CAPS[t]
        out[base:base + ncnt] = np.maximum(out_acc[:ncnt], 0.0)
    return out


def emulate(emb_table, weights, neighbors, relations):
    in_maps, perms = prep(emb_table, weights, neighbors, relations)
    full = np.zeros((N, D), dtype=np.float32)
    for c in range(NCORES):
        full[perms[c]] = emulate_core(in_maps[c])
    return full


# ---------------------------------------------------------------------------
# Bass program
# ---------------------------------------------------------------------------

def build_program():
    import concourse.bacc as bacc
    import concourse.tile as tile
    import concourse.mybir as mybir

    nc = bacc.Bacc(
        "TRN2", target_bir_lowering=False, debug=False,
        num_devices=NCORES,
    )
    BF = mybir.dt.bfloat16
    F32 = mybir.dt.float32
    I16 = mybir.dt.int16
    F8 = mybir.dt.float8e4

    emb = nc.dram_tensor("emb", [UMAX, D], BF, kind="ExternalInput").ap()
    wsb = nc.dram_tensor("wsb", [128, R1 * 2 * D], BF, kind="ExternalInput").ap()
    idx = nc.dram_tensor("idx", [NTILES * 128, IDXW], I16, kind="ExternalInput").ap()
    sel = nc.dram_tensor("sel", [NTILES * 128, NSLOT], F8,
                         kind="ExternalInput").ap()
    out = nc.dram_tensor("out", [NPC, D], F32, kind="ExternalOutput").ap()

    Relu = mybir.ActivationFunctionType.Relu

    with tile.TileContext(nc) as tc:
        with (
            tc.tile_pool(name="wpool", bufs=1) as wpool,
            tc.tile_pool(name="etpool", bufs=PF + 1) as etpool,
            tc.tile_pool(name="selpool", bufs=PF + 1) as selpool,
            tc.tile_pool(name="idxpool", bufs=PF + 1) as idxpool,
            tc.tile_pool(name="ypool", bufs=2 * (SKEW + 1)) as ypool,
            tc.tile_pool(name="opool", bufs=2) as opool,
            tc.tile_pool(name="psy", bufs=SKEW + 1, space="PSUM") as psy,
            tc.tile_pool(name="pso", bufs=1, space="PSUM") as pso,
        ):
            wts = {}

            def load_w(r0, r1):
                wtg = wpool.tile([128, (r1 - r0) * 2 * D], BF,
                                 name=f"wt{r0}", uniquify=False)
                nc.sync.dma_start(out=wtg[:],
                                  in_=wsb[:, r0 * 2 * D:r1 * 2 * D])
                for r in range(r0, r1):
                    wts[r] = (wtg, r0)

            ets, sels = {}, {}

            def pre_gather(t):
                idx_t = idxpool.tile([128, IDXW], I16, name="idx_t")
                nc.sync.dma_start(
                    out=idx_t[:], in_=idx[t * 128:(t + 1) * 128, :])
                segs, col = [], 0
                for gi, (a, b) in enumerate(GSEG):
                    n = b - a
                    eth = etpool.tile([128, 2, n], BF, name=f"et{gi}")
                    nc.gpsimd.dma_gather(
                        out_ap=eth[:],
                        in_ap=emb,
                        idxs_ap=idx_t[:, col:col + n // 16],
                        num_idxs=n,
                        num_idxs_reg=n,
                        elem_size=D,
                        transpose=True,
                        single_packet=False,
                    )
                    col += n // 16
                    segs.append(eth)
                ets[t] = segs

            def pre_sel(t):
                sel_t = selpool.tile([128, NSLOT], F8, name="sel_t")
                nc.sync.dma_start(
                    out=sel_t[:], in_=sel[t * 128:(t + 1) * 128, :])
                sels[t] = sel_t

            def prefetch(t):
                if t >= NTILES:
                    return
                pre_gather(t)
                pre_sel(t)

            # startup orchestration: gather(0) first, early W chunks, sel(0),
            # then stream the rest so the PE can start by ~9us.
            # PE warm-up: keeps the PE busy from t~0.6us so the p-state
            # ramp completes during the DMA pipeline fill (results unused)
            dumt = wpool.tile([128, 512], BF, name="dumt")
            nc.vector.memset(dumt[:], 0)
            dump = psy.tile([128, D], F32, name="yp", uniquify=False)
            for i in range(20):
                nc.tensor.matmul(out=dump[:], lhsT=dumt[:, :128],
                                 rhs=dumt[:, 256:256 + D], start=True,
                                 stop=True, skip_group_check=True)

            pre_gather(0)
            load_w(0, 3)
            load_w(3, 7)
            pre_sel(0)
            load_w(7, 12)
            pre_gather(1)
            load_w(12, 17)
            pre_sel(1)
            load_w(17, 22)
            prefetch(2)
            load_w(22, 27)
            load_w(27, R1)

            for t in range(NTILES):
                segs, sel_t = ets.pop(t), sels.pop(t)
                outp = pso.tile([128, D], F32)
                ys = [None] * R1
                for r in range(R1 + SKEW):
                    if r < R1:
                        yp = psy.tile([128, D], F32, name="yp")
                        gi = next(i for i, (a, b) in enumerate(GSEG)
                                  if a <= r * P < b)
                        eth, off = segs[gi], r * P - GSEG[gi][0]
                        wtg, rbase = wts[r]
                        for c in range(2):
                            a = ((r - rbase) * 2 + c) * D
                            nc.tensor.matmul(
                                out=yp[:],
                                lhsT=eth[:, c, off:off + P],
                                rhs=wtg[:, a:a + D],
                                start=(c == 0), stop=(c == 1),
                            )
                        ysb = ypool.tile([128, D], BF, name="ysb")
                        if r % 2 == 0:
                            nc.vector.tensor_copy(out=ysb[:], in_=yp[:])
                        else:
                            nc.scalar.copy(out=ysb[:], in_=yp[:])
                        ys[r] = ysb
                    if r >= SKEW:
                        q = r - SKEW
                        nc.tensor.matmul(
                            out=outp[:],
                            lhsT=sel_t[:, q * P:(q + 1) * P],
                            rhs=ys[q][:],
                            start=(q == 0), stop=(q == R1 - 1),
                        )
                prefetch(t + PF)
                osb = opool.tile([128, D], F32)
                nc.scalar.activation(out=osb[:], in_=outp[:], func=Relu)
                base, ncnt = ROW_BASE[t], CAPS[t]
                nc.sync.dma_start(
                    out=out[base:base + ncnt, :], in_=osb[:ncnt, :])

    nc.compile()
    return nc


_NC_CACHE = []


def _get_program():
    if not _NC_CACHE:
        _NC_CACHE.append(build_program())
    return _NC_CACHE[0]


# ---------------------------------------------------------------------------
# Entry point
# ---------------------------------------------------------------------------

def kernel(emb_table, weights, neighbors, relations):
    from concourse import bass_utils

    in_maps, perms = prep(emb_table, weights, neighbors, relations)
    nc = _get_program()
    res = bass_utils.run_bass_kernel_spmd(
        nc, in_maps, core_ids=list(range(NCORES)),
    )
    full = np.zeros((N, D), dtype=np.float32)
    for c in range(NCORES):
        full[perms[c]] = res.results[c]["out"]
    return full



# revision 39
# speedup vs baseline: 1.0452x; 1.0113x over previous
"""Trainium2 Bass kernel for LoopRelationalGraphConvolution.

Math (matches the jax reference):
    out[n] = relu( SCALE * sum_s  W[rel[n,s]] @ emb[neighbors[n,s]] )
    SCALE  = 1000 / (R1 * S)      (folds the mean over S and the /R1 * 1000)

Design (8 NeuronCores, data-parallel over the 8192-node batch):
  Each core owns 1024 nodes, split into 9 node-tiles ([114]*8 + [112] nodes)
  chosen by a host-side balancer so that every (tile, relation) bucket has
  <=128 edges.  Per tile the device kernel:
    1. dma_gather(transpose=True): fetches the tile's 33*128 edge-slot
       embeddings (bf16, compacted per-core int16 ids) with the embedding dim
       landing on SBUF partitions:  ET[p, c, i] = emb[idx_i][c*128+p].
    2. stage-1 matmuls: per relation-chunk r (128 slots), in 2 K-chunks over D:
       Y[slot, o] += ET[:, c, slots]^T @ W_r[c]       (PSUM, f32)
    3. stage-2 matmul: 0/1 selection matrix reduces edge slots into node rows:
       out_psum[node, o] += SEL_r^T @ Y_bf16          (accumulated over all r)
    4. relu on PSUM->SBUF evacuation, DMA node rows to DRAM.
  The device program is fully static and identical across cores (SPMD); all
  data-dependence lives in the index / selection arrays.  Host post-step
  inverse-permutes rows back to the original node order.
"""

import numpy as np
import ml_dtypes

bf16 = ml_dtypes.bfloat16
fp8 = ml_dtypes.float8_e4m3

# Problem constants (hardcoded per contract).
V = 100000
D = 256
R1 = 33          # relations incl. self-loop
N = 8192
S = 32
NCORES = 8
NPC = N // NCORES          # 1024 nodes per core
NTILES = 9                 # node-tiles per core
CAPS = [114] * 8 + [112]   # nodes per tile (uniform across cores)
ROW_BASE = np.concatenate([[0], np.cumsum(CAPS)]).tolist()
P = 128
NSLOT = R1 * P             # 4224 edge slots per tile
GSPLIT = [0, 2, 6, 12, 22, R1]   # gather segment chunk boundaries
GSEG = [(a * P, b * P) for a, b in zip(GSPLIT, GSPLIT[1:])]
IDXW = NSLOT // 16         # 264 int16 idx columns (16-partition wrap)
UMAX = 32768               # compacted per-core embedding rows (int16 limit)
SCALE = 1000.0 / (R1 * S)

# Software-pipeline skew between stage-1 and stage-2 of consecutive chunks,
# so the PE never stalls on the PSUM->SBUF copy of the current chunk.
SKEW = 6
PF = 3      # tile prefetch depth


# ---------------------------------------------------------------------------
# Host-side preparation
# ---------------------------------------------------------------------------

def _balance_tiles(hist):
    """Assign NPC nodes to NTILES tiles (exactly CAPS[t] nodes each),
    minimizing the max per-(tile, relation) edge count. hist: [NPC, R1].
    Greedy: hardest nodes first, place on the tile minimizing the resulting
    peak bucket."""
    order = np.argsort(-hist.max(axis=1), kind="stable")
    loads = np.zeros((NTILES, R1), dtype=np.int64)
    counts = np.zeros(NTILES, dtype=np.int64)
    tiles = [[] for _ in range(NTILES)]
    for n in order:
        h = hist[n]
        best_t, best_key = -1, None
        for t in range(NTILES):
            if counts[t] >= CAPS[t]:
                continue
            new = loads[t] + h
            key = (int(new.max()), int(loads[t].max()), int(new.sum()))
            if best_key is None or key < best_key:
                best_key, best_t = key, t
        tiles[best_t].append(int(n))
        loads[best_t] += h
        counts[best_t] += 1
    return tiles, loads


def prep(emb_table, weights, neighbors, relations):
    """Build per-core device arrays. Returns (in_maps, perms)."""
    emb_bf = np.asarray(emb_table).astype(bf16)
    w = np.asarray(weights, dtype=np.float32) * SCALE         # [R1, D_out, D_in]
    # W_sb[p, (r*2 + c)*D + o] = w[r, o, c*128+p]
    w_rdo = np.ascontiguousarray(w.transpose(0, 2, 1))        # [r, d, o]
    W_sb = np.ascontiguousarray(
        w_rdo.reshape(R1, 2, 128, D).transpose(2, 0, 1, 3)    # [p, r, c, o]
    ).reshape(128, R1 * 2 * D).astype(bf16)

    neighbors = np.asarray(neighbors).astype(np.int64)
    relations = np.asarray(relations).astype(np.int64)

    in_maps, perms = [], []
    for c in range(NCORES):
        nb = neighbors[c * NPC:(c + 1) * NPC]                 # [NPC, S]
        rel = relations[c * NPC:(c + 1) * NPC]
        uniq, inv = np.unique(nb.ravel(), return_inverse=True)
        inv = inv.reshape(nb.shape).astype(np.int64)
        U = len(uniq)
        assert U <= UMAX, U
        emb_c = np.zeros((UMAX, D), dtype=bf16)
        emb_c[:U] = emb_bf[uniq]

        hist = np.zeros((NPC, R1), dtype=np.int64)
        np.add.at(hist, (np.repeat(np.arange(NPC), S), rel.ravel()), 1)
        tiles, loads = _balance_tiles(hist)
        assert loads.max() <= P, f"balance failed: max bucket {loads.max()}"

        idx_all = np.zeros((NTILES, 128, IDXW), dtype=np.int16)
        sel_all = np.zeros((NTILES, 128, NSLOT), dtype=fp8)
        perm = []
        for t, nodes in enumerate(tiles):
            nodes = np.array(nodes, dtype=np.int64)
            ncnt = len(nodes)
            assert ncnt == CAPS[t]
            perm.extend((c * NPC + nodes).tolist())
            # edges of this tile
            er = rel[nodes].ravel()                            # relation per edge
            ei = inv[nodes].ravel()                            # compact nbr id
            ej = np.repeat(np.arange(ncnt), S)                 # local node idx
            order = np.argsort(er, kind="stable")
            er_s, ei_s, ej_s = er[order], ei[order], ej[order]
            # position within relation group
            start = np.searchsorted(er_s, np.arange(R1))
            pos = np.arange(ncnt * S) - start[er_s]
            slot = er_s * P + pos                              # [ncnt*S]
            slots_idx = np.zeros(NSLOT, dtype=np.int16)
            slots_idx[slot] = ei_s
            sel = np.zeros((NSLOT, 128), dtype=fp8)
            sel[slot, ej_s] = 1.0
            # idx wrap per gather segment: idx i at partition i%16, col i//16
            wrapped = np.concatenate(
                [slots_idx[a:b].reshape((b - a) // 16, 16).T
                 for a, b in GSEG], axis=1)                    # [16, IDXW]
            idx_all[t] = np.tile(wrapped, (8, 1))
            # device SEL layout: [part p = slot-in-chunk, free = r*ncnt+node]
            packed = np.ascontiguousarray(
                sel.reshape(R1, P, 128).transpose(1, 0, 2)[:, :, :ncnt]
            ).reshape(P, R1 * ncnt)
            sel_all[t, :, :R1 * ncnt] = packed
        in_maps.append({
            "emb": emb_c,
            "wsb": W_sb,
            "idx": np.ascontiguousarray(idx_all.reshape(NTILES * 128, IDXW)),
            "sel": np.ascontiguousarray(sel_all.reshape(NTILES * 128, NSLOT)),
        })
        perms.append(np.array(perm, dtype=np.int64))

    return in_maps, perms


# ---------------------------------------------------------------------------
# Numpy emulation (bf16-faithful) for validation
# ---------------------------------------------------------------------------

def emulate_core(in_map):
    emb = in_map["emb"]                                        # [UMAX, D] bf16
    wsb = in_map["wsb"].reshape(128, R1, 2, D)                 # [p, r, c, o]
    idx = in_map["idx"].reshape(NTILES, 128, IDXW)
    sel = in_map["sel"].reshape(NTILES, 128, NSLOT)
    out = np.zeros((NPC, D), dtype=np.float32)
    for t in range(NTILES):
        parts, col = [], 0
        for a, b in GSEG:
            w = (b - a) // 16
            parts.append(idx[t, :16, col:col + w].T.reshape(b - a))
            col += w
        slots_idx = np.concatenate(parts)                      # unwrap
        X = emb[slots_idx]                                     # [NSLOT, D] bf16
        base, ncnt = ROW_BASE[t], CAPS[t]
        out_acc = np.zeros((ncnt, D), dtype=np.float32)
        for r in range(R1):
            Xr = X[r * P:(r + 1) * P].astype(np.float32)       # [128, D]
            Y = (Xr[:, :128] @ wsb[:, r, 0, :].astype(np.float32)
                 + Xr[:, 128:] @ wsb[:, r, 1, :].astype(np.float32))
            Yb = Y.astype(bf16).astype(np.float32)             # PSUM->SBUF bf16
            selr = sel[t][:, r * ncnt:(r + 1) * ncnt].astype(np.float32)
            out_acc += selr.T @ Yb
        res = np.maximum(out_acc, 0.0)
        out[base:base + ncnt] = res.astype(bf16).astype(np.float32)
    return out


def emulate(emb_table, weights, neighbors, relations):
    in_maps, perms = prep(emb_table, weights, neighbors, relations)
    full = np.zeros((N, D), dtype=np.float32)
    for c in range(NCORES):
        full[perms[c]] = emulate_core(in_maps[c])
    return full


# ---------------------------------------------------------------------------
# Bass program
# ---------------------------------------------------------------------------

def build_program():
    import concourse.bacc as bacc
    import concourse.tile as tile
    import concourse.mybir as mybir

    nc = bacc.Bacc(
        "TRN2", target_bir_lowering=False, debug=False,
        num_devices=NCORES,
    )
    BF = mybir.dt.bfloat16
    F32 = mybir.dt.float32
    I16 = mybir.dt.int16
    F8 = mybir.dt.float8e4

    emb = nc.dram_tensor("emb", [UMAX, D], BF, kind="ExternalInput").ap()
    wsb = nc.dram_tensor("wsb", [128, R1 * 2 * D], BF, kind="ExternalInput").ap()
    idx = nc.dram_tensor("idx", [NTILES * 128, IDXW], I16, kind="ExternalInput").ap()
    sel = nc.dram_tensor("sel", [NTILES * 128, NSLOT], F8,
                         kind="ExternalInput").ap()
    out = nc.dram_tensor("out", [NPC, D], BF, kind="ExternalOutput").ap()

    Relu = mybir.ActivationFunctionType.Relu

    with tile.TileContext(nc) as tc:
        with (
            tc.tile_pool(name="wpool", bufs=1) as wpool,
            tc.tile_pool(name="etpool", bufs=PF + 1) as etpool,
            tc.tile_pool(name="selpool", bufs=PF + 1) as selpool,
            tc.tile_pool(name="idxpool", bufs=PF + 1) as idxpool,
            tc.tile_pool(name="ypool", bufs=2 * (SKEW + 1)) as ypool,
            tc.tile_pool(name="opool", bufs=2) as opool,
            tc.tile_pool(name="psy", bufs=SKEW, space="PSUM") as psy,
            tc.tile_pool(name="pso", bufs=2, space="PSUM") as pso,
        ):
            wts = {}

            def load_w(r0, r1):
                wtg = wpool.tile([128, (r1 - r0) * 2 * D], BF,
                                 name=f"wt{r0}", uniquify=False)
                nc.sync.dma_start(out=wtg[:],
                                  in_=wsb[:, r0 * 2 * D:r1 * 2 * D])
                for r in range(r0, r1):
                    wts[r] = (wtg, r0)

            ets, sels = {}, {}

            def pre_gather(t):
                idx_t = idxpool.tile([128, IDXW], I16, name="idx_t")
                nc.sync.dma_start(
                    out=idx_t[:], in_=idx[t * 128:(t + 1) * 128, :])
                segs, col = [], 0
                for gi, (a, b) in enumerate(GSEG):
                    n = b - a
                    eth = etpool.tile([128, 2, n], BF, name=f"et{gi}")
                    nc.gpsimd.dma_gather(
                        out_ap=eth[:],
                        in_ap=emb,
                        idxs_ap=idx_t[:, col:col + n // 16],
                        num_idxs=n,
                        num_idxs_reg=n,
                        elem_size=D,
                        transpose=True,
                        single_packet=False,
                    )
                    col += n // 16
                    segs.append(eth)
                ets[t] = segs

            def pre_sel(t):
                sw = R1 * CAPS[t]
                sel_t = selpool.tile([128, NSLOT], F8, name="sel_t")
                nc.sync.dma_start(
                    out=sel_t[:, :sw], in_=sel[t * 128:(t + 1) * 128, :sw])
                sels[t] = sel_t

            def prefetch(t):
                if t >= NTILES:
                    return
                pre_gather(t)
                pre_sel(t)

            # startup orchestration: gather(0) first, early W chunks, sel(0),
            # then stream the rest so the PE can start by ~9us.
            # PE warm-up: keeps the PE busy from t~0.6us so the p-state
            # ramp completes during the DMA pipeline fill (results unused)
            dumt = wpool.tile([128, 512], BF, name="dumt")
            nc.vector.memset(dumt[:], 0)
            dump = psy.tile([128, D], F32, name="yp", uniquify=False)
            for i in range(20):
                nc.tensor.matmul(out=dump[:], lhsT=dumt[:, :128],
                                 rhs=dumt[:, 256:256 + D], start=True,
                                 stop=True, skip_group_check=True)

            pre_gather(0)
            load_w(0, 3)
            load_w(3, 7)
            pre_sel(0)
            load_w(7, 12)
            pre_gather(1)
            load_w(12, 17)
            pre_sel(1)
            load_w(17, 22)
            prefetch(2)
            load_w(22, 27)
            load_w(27, R1)

            for t in range(NTILES):
                segs, sel_t = ets.pop(t), sels.pop(t)
                outp = pso.tile([128, D], F32)
                ys = [None] * R1
                for r in range(R1 + SKEW):
                    if r < R1:
                        yp = psy.tile([128, D], F32, name="yp")
                        gi = next(i for i, (a, b) in enumerate(GSEG)
                                  if a <= r * P < b)
                        eth, off = segs[gi], r * P - GSEG[gi][0]
                        wtg, rbase = wts[r]
                        for c in range(2):
                            a = ((r - rbase) * 2 + c) * D
                            nc.tensor.matmul(
                                out=yp[:],
                                lhsT=eth[:, c, off:off + P],
                                rhs=wtg[:, a:a + D],
                                start=(c == 0), stop=(c == 1),
                            )
                        ysb = ypool.tile([128, D], BF, name="ysb")
                        if r % 2 == 0:
                            nc.vector.tensor_copy(out=ysb[:], in_=yp[:])
                        else:
                            nc.scalar.copy(out=ysb[:], in_=yp[:])
                        ys[r] = ysb
                    if r >= SKEW:
                        q = r - SKEW
                        ncnt = CAPS[t]
                        nc.tensor.matmul(
                            out=outp[:ncnt, :],
                            lhsT=sel_t[:, q * ncnt:(q + 1) * ncnt],
                            rhs=ys[q][:],
                            start=(q == 0), stop=(q == R1 - 1),
                        )
                prefetch(t + PF)
                ncnt = CAPS[t]
                osb = opool.tile([128, D], BF)
                nc.scalar.activation(out=osb[:ncnt, :], in_=outp[:ncnt, :],
                                     func=Relu)
                base = ROW_BASE[t]
                nc.sync.dma_start(
                    out=out[base:base + ncnt, :], in_=osb[:ncnt, :])

    nc.compile()
    return nc


_NC_CACHE = []


def _get_program():
    if not _NC_CACHE:
        _NC_CACHE.append(build_program())
    return _NC_CACHE[0]


# ---------------------------------------------------------------------------
# Entry point
# ---------------------------------------------------------------------------

def kernel(emb_table, weights, neighbors, relations):
    from concourse import bass_utils

    in_maps, perms = prep(emb_table, weights, neighbors, relations)
    nc = _get_program()
    res = bass_utils.run_bass_kernel_spmd(
        nc, in_maps, core_ids=list(range(NCORES)),
    )
    full = np.zeros((N, D), dtype=np.float32)
    for c in range(NCORES):
        full[perms[c]] = res.results[c]["out"].astype(np.float32)
    return full



# revision 45
# speedup vs baseline: 1.1062x; 1.0584x over previous
"""Trainium2 Bass kernel for LoopRelationalGraphConvolution.

Math (matches the jax reference):
    out[n] = relu( SCALE * sum_s  W[rel[n,s]] @ emb[neighbors[n,s]] )
    SCALE  = 1000 / (R1 * S)      (folds the mean over S and the /R1 * 1000)

Design (8 NeuronCores, data-parallel over the 8192-node batch):
  Each core owns 1024 nodes, split into 9 node-tiles ([114]*8 + [112] nodes)
  chosen by a host-side balancer so that every (tile, relation) bucket has
  <=128 edges.  Per tile the device kernel:
    1. dma_gather(transpose=True): fetches the tile's 33*128 edge-slot
       embeddings (bf16, compacted per-core int16 ids) with the embedding dim
       landing on SBUF partitions:  ET[p, c, i] = emb[idx_i][c*128+p].
    2. stage-1 matmuls: per relation-chunk r (128 slots), in 2 K-chunks over D:
       Y[slot, o] += ET[:, c, slots]^T @ W_r[c]       (PSUM, f32)
    3. stage-2 matmul: 0/1 selection matrix reduces edge slots into node rows:
       out_psum[node, o] += SEL_r^T @ Y_bf16          (accumulated over all r)
    4. relu on PSUM->SBUF evacuation, DMA node rows to DRAM.
  The device program is fully static and identical across cores (SPMD); all
  data-dependence lives in the index / selection arrays.  Host post-step
  inverse-permutes rows back to the original node order.
"""

import numpy as np
import ml_dtypes

bf16 = ml_dtypes.bfloat16
fp8 = ml_dtypes.float8_e4m3

# Problem constants (hardcoded per contract).
V = 100000
D = 256
R1 = 33          # relations incl. self-loop
N = 8192
S = 32
NCORES = 8
NPC = N // NCORES          # 1024 nodes per core
NTILES = 9                 # node-tiles per core
CAPS = [114] * 8 + [112]   # nodes per tile (uniform across cores)
ROW_BASE = np.concatenate([[0], np.cumsum(CAPS)]).tolist()
P = 128
NSLOT = R1 * P             # 4224 edge slots per tile
GSPLIT = [0, 2, 6, 12, 22, R1]   # gather segment chunk boundaries
GSEG = [(a * P, b * P) for a, b in zip(GSPLIT, GSPLIT[1:])]
FK = 22     # relations 0..FK-1 run stage-1 in fp8 DoubleRow (two-level fp8,
            # 3 products at 0.5 cyc/row); the rest stay bf16. FK must align
            # with a GSPLIT boundary so each gather segment is one mode.
WA = 16.0   # pre-scale on W so fp8 residuals stay in e4m3 normal range;
            # folded out in the final relu
IDXW = NSLOT // 16         # 264 int16 idx columns (16-partition wrap)
UMAX = 32768               # compacted per-core embedding rows (int16 limit)
SCALE = 1000.0 / (R1 * S)

# Software-pipeline skew between stage-1 and stage-2 of consecutive chunks,
# so the PE never stalls on the PSUM->SBUF copy of the current chunk.
SKEW = 6
PF = 3      # tile prefetch depth


# ---------------------------------------------------------------------------
# Host-side preparation
# ---------------------------------------------------------------------------

def _balance_tiles(hist):
    """Assign NPC nodes to NTILES tiles (exactly CAPS[t] nodes each),
    minimizing the max per-(tile, relation) edge count. hist: [NPC, R1].
    Greedy: hardest nodes first, place on the tile minimizing the resulting
    peak bucket."""
    order = np.argsort(-hist.max(axis=1), kind="stable")
    loads = np.zeros((NTILES, R1), dtype=np.int64)
    counts = np.zeros(NTILES, dtype=np.int64)
    tiles = [[] for _ in range(NTILES)]
    for n in order:
        h = hist[n]
        best_t, best_key = -1, None
        for t in range(NTILES):
            if counts[t] >= CAPS[t]:
                continue
            new = loads[t] + h
            key = (int(new.max()), int(loads[t].max()), int(new.sum()))
            if best_key is None or key < best_key:
                best_key, best_t = key, t
        tiles[best_t].append(int(n))
        loads[best_t] += h
        counts[best_t] += 1
    return tiles, loads


def _split2(x):
    """Two-level fp8 decomposition: x ~= hi + lo, both fp8_e4m3."""
    x = np.asarray(x, dtype=np.float32)
    hi = x.astype(fp8)
    lo = (x - hi.astype(np.float32)).astype(fp8)
    return hi, lo


def prep(emb_table, weights, neighbors, relations):
    """Build per-core device arrays. Returns (in_maps, perms)."""
    emb32 = np.asarray(emb_table, dtype=np.float32)
    emb_bf = emb32.astype(bf16)
    w = np.asarray(weights, dtype=np.float32) * WA            # [R1, D_out, D_in]
    # bf16 part: W16_sb[p, (r*2 + c)*D + o] = w[r, o, c*128+p],  r >= FK
    w_rdo = np.ascontiguousarray(w.transpose(0, 2, 1))        # [r, d, o]
    W16_sb = np.ascontiguousarray(
        w_rdo[FK:].reshape(R1 - FK, 2, 128, D).transpose(2, 0, 1, 3)
    ).reshape(128, (R1 - FK) * 2 * D).astype(bf16)
    # fp8 part: two-level; W8_sb[p, (((r*2+lvl)*2+c)*D + o)] = W{lvl}[r, c*128+p, o]
    wh, wl = _split2(w_rdo[:FK])
    W8_sb = np.ascontiguousarray(
        np.stack([wh, wl], axis=1).reshape(FK, 2, 2, 128, D)  # [r,lvl,c,p,o]
        .transpose(3, 0, 1, 2, 4)).reshape(128, FK * 2 * 2 * D)

    neighbors = np.asarray(neighbors).astype(np.int64)
    relations = np.asarray(relations).astype(np.int64)

    in_maps, perms = [], []
    for c in range(NCORES):
        nb = neighbors[c * NPC:(c + 1) * NPC]                 # [NPC, S]
        rel = relations[c * NPC:(c + 1) * NPC]
        uniq, inv = np.unique(nb.ravel(), return_inverse=True)
        inv = inv.reshape(nb.shape).astype(np.int64)
        U = len(uniq)
        assert U <= UMAX, U
        emb_c = np.zeros((UMAX, D), dtype=bf16)
        emb_c[:U] = emb_bf[uniq]
        hi, lo = _split2(emb32[uniq])
        ehl = np.zeros((UMAX, 2 * D), dtype=fp8)              # [hi|lo] pairs
        ehl[:U, 0::2] = hi
        ehl[:U, 1::2] = lo

        hist = np.zeros((NPC, R1), dtype=np.int64)
        np.add.at(hist, (np.repeat(np.arange(NPC), S), rel.ravel()), 1)
        tiles, loads = _balance_tiles(hist)
        assert loads.max() <= P, f"balance failed: max bucket {loads.max()}"

        idx_all = np.zeros((NTILES, 128, IDXW), dtype=np.int16)
        sel_all = np.zeros((NTILES, 128, NSLOT), dtype=fp8)
        perm = []
        for t, nodes in enumerate(tiles):
            nodes = np.array(nodes, dtype=np.int64)
            ncnt = len(nodes)
            assert ncnt == CAPS[t]
            perm.extend((c * NPC + nodes).tolist())
            # edges of this tile
            er = rel[nodes].ravel()                            # relation per edge
            ei = inv[nodes].ravel()                            # compact nbr id
            ej = np.repeat(np.arange(ncnt), S)                 # local node idx
            order = np.argsort(er, kind="stable")
            er_s, ei_s, ej_s = er[order], ei[order], ej[order]
            # position within relation group
            start = np.searchsorted(er_s, np.arange(R1))
            pos = np.arange(ncnt * S) - start[er_s]
            slot = er_s * P + pos                              # [ncnt*S]
            slots_idx = np.zeros(NSLOT, dtype=np.int16)
            slots_idx[slot] = ei_s
            sel = np.zeros((NSLOT, 128), dtype=fp8)
            sel[slot, ej_s] = 1.0
            # idx wrap per gather segment: idx i at partition i%16, col i//16
            wrapped = np.concatenate(
                [slots_idx[a:b].reshape((b - a) // 16, 16).T
                 for a, b in GSEG], axis=1)                    # [16, IDXW]
            idx_all[t] = np.tile(wrapped, (8, 1))
            # device SEL layout: [part p = slot-in-chunk, free = r*ncnt+node]
            packed = np.ascontiguousarray(
                sel.reshape(R1, P, 128).transpose(1, 0, 2)[:, :, :ncnt]
            ).reshape(P, R1 * ncnt)
            sel_all[t, :, :R1 * ncnt] = packed
        in_maps.append({
            "emb": emb_c,
            "emb8": ehl,
            "wsb16": W16_sb,
            "wsb8": W8_sb,
            "idx": np.ascontiguousarray(idx_all.reshape(NTILES * 128, IDXW)),
            "sel": np.ascontiguousarray(sel_all.reshape(NTILES * 128, NSLOT)),
        })
        perms.append(np.array(perm, dtype=np.int64))

    return in_maps, perms


# ---------------------------------------------------------------------------
# Numpy emulation (bf16-faithful) for validation
# ---------------------------------------------------------------------------

def emulate_core(in_map):
    emb = in_map["emb"]                                        # [UMAX, D] bf16
    xh = in_map["emb8"][:, 0::2].astype(np.float32)
    xl = in_map["emb8"][:, 1::2].astype(np.float32)
    w16 = in_map["wsb16"].reshape(128, R1 - FK, 2, D)          # [p, r, c, o]
    w8 = in_map["wsb8"].reshape(128, FK, 2, 2, D)              # [p, r, lvl, c, o]
    w8f = np.ascontiguousarray(w8.transpose(1, 2, 3, 0, 4)).reshape(
        FK, 2, 2 * 128, D).astype(np.float32)                  # [r, lvl, d, o]
    idx = in_map["idx"].reshape(NTILES, 128, IDXW)
    sel = in_map["sel"].reshape(NTILES, 128, NSLOT)
    out = np.zeros((NPC, D), dtype=np.float32)
    for t in range(NTILES):
        parts, col = [], 0
        for a, b in GSEG:
            w = (b - a) // 16
            parts.append(idx[t, :16, col:col + w].T.reshape(b - a))
            col += w
        slots_idx = np.concatenate(parts)                      # unwrap
        base, ncnt = ROW_BASE[t], CAPS[t]
        out_acc = np.zeros((ncnt, D), dtype=np.float32)
        for r in range(R1):
            sl = slots_idx[r * P:(r + 1) * P]
            if r < FK:
                Y = (xh[sl] @ w8f[r, 0] + xl[sl] @ w8f[r, 0]
                     + xh[sl] @ w8f[r, 1])
            else:
                Xr = emb[sl].astype(np.float32)                # [128, D]
                rr = r - FK
                Y = (Xr[:, :128] @ w16[:, rr, 0, :].astype(np.float32)
                     + Xr[:, 128:] @ w16[:, rr, 1, :].astype(np.float32))
            Yb = Y.astype(bf16).astype(np.float32)             # PSUM->SBUF bf16
            selr = sel[t][:, r * ncnt:(r + 1) * ncnt].astype(np.float32)
            out_acc += selr.T @ Yb
        res = np.maximum(out_acc * (SCALE / WA), 0.0)
        out[base:base + ncnt] = res.astype(bf16).astype(np.float32)
    return out


def emulate(emb_table, weights, neighbors, relations):
    in_maps, perms = prep(emb_table, weights, neighbors, relations)
    full = np.zeros((N, D), dtype=np.float32)
    for c in range(NCORES):
        full[perms[c]] = emulate_core(in_maps[c])
    return full


# ---------------------------------------------------------------------------
# Bass program
# ---------------------------------------------------------------------------

def build_program():
    import concourse.bacc as bacc
    import concourse.tile as tile
    import concourse.mybir as mybir

    nc = bacc.Bacc(
        "TRN2", target_bir_lowering=False, debug=False,
        num_devices=NCORES,
    )
    BF = mybir.dt.bfloat16
    F32 = mybir.dt.float32
    I16 = mybir.dt.int16
    F8 = mybir.dt.float8e4
    DR = mybir.MatmulPerfMode.DoubleRow

    emb = nc.dram_tensor("emb", [UMAX, D], BF, kind="ExternalInput").ap()
    emb8 = nc.dram_tensor("emb8", [UMAX, 2 * D], F8,
                          kind="ExternalInput").ap()
    wsb16 = nc.dram_tensor("wsb16", [128, (R1 - FK) * 2 * D], BF,
                           kind="ExternalInput").ap()
    wsb8 = nc.dram_tensor("wsb8", [128, FK * 2 * 2 * D], F8,
                          kind="ExternalInput").ap()
    idx = nc.dram_tensor("idx", [NTILES * 128, IDXW], I16, kind="ExternalInput").ap()
    sel = nc.dram_tensor("sel", [NTILES * 128, NSLOT], F8,
                         kind="ExternalInput").ap()
    out = nc.dram_tensor("out", [NPC, D], BF, kind="ExternalOutput").ap()

    Relu = mybir.ActivationFunctionType.Relu

    with tile.TileContext(nc) as tc:
        with (
            tc.tile_pool(name="wpool", bufs=1) as wpool,
            tc.tile_pool(name="etpool", bufs=PF + 1) as etpool,
            tc.tile_pool(name="selpool", bufs=PF + 1) as selpool,
            tc.tile_pool(name="idxpool", bufs=PF + 1) as idxpool,
            tc.tile_pool(name="ypool", bufs=2 * (SKEW + 1)) as ypool,
            tc.tile_pool(name="opool", bufs=2) as opool,
            tc.tile_pool(name="psy", bufs=SKEW, space="PSUM") as psy,
            tc.tile_pool(name="pso", bufs=2, space="PSUM") as pso,
        ):
            wts = {}

            def load_w(r0, r1):
                # [r0, r1) must lie entirely on one side of FK
                if r1 <= FK:
                    wtg = wpool.tile([128, r1 - r0, 2, 2, D], F8,
                                     name=f"wt{r0}", uniquify=False)
                    nc.sync.dma_start(
                        out=wtg[:],
                        in_=wsb8[:, r0 * 4 * D:r1 * 4 * D])
                else:
                    a, b = r0 - FK, r1 - FK
                    wtg = wpool.tile([128, (r1 - r0) * 2 * D], BF,
                                     name=f"wt{r0}", uniquify=False)
                    nc.sync.dma_start(out=wtg[:],
                                      in_=wsb16[:, a * 2 * D:b * 2 * D])
                for r in range(r0, r1):
                    wts[r] = (wtg, r0)

            ets, sels = {}, {}

            def pre_gather(t):
                idx_t = idxpool.tile([128, IDXW], I16, name="idx_t")
                nc.sync.dma_start(
                    out=idx_t[:], in_=idx[t * 128:(t + 1) * 128, :])
                segs, col = [], 0
                for gi, (a, b) in enumerate(GSEG):
                    n = b - a
                    if b <= FK * P:
                        eth = etpool.tile([128, 4, n], F8, name=f"et{gi}")
                        nc.gpsimd.dma_gather(
                            out_ap=eth[:],
                            in_ap=emb8,
                            idxs_ap=idx_t[:, col:col + n // 16],
                            num_idxs=n,
                            num_idxs_reg=n,
                            elem_size=2 * D,
                            transpose=True,
                            single_packet=False,
                        )
                        # true byte layout: [p][ktile c:2][slot:n][hi/lo b:2]
                        eth = eth[:].rearrange("p f n -> p (f n)").rearrange(
                            "p (c i b) -> p c i b", c=2, i=n, b=2)
                    else:
                        eth = etpool.tile([128, 2, n], BF, name=f"et{gi}")
                        nc.gpsimd.dma_gather(
                            out_ap=eth[:],
                            in_ap=emb,
                            idxs_ap=idx_t[:, col:col + n // 16],
                            num_idxs=n,
                            num_idxs_reg=n,
                            elem_size=D,
                            transpose=True,
                            single_packet=False,
                        )
                    col += n // 16
                    segs.append(eth)
                ets[t] = segs

            def pre_sel(t):
                sw = R1 * CAPS[t]
                sel_t = selpool.tile([128, NSLOT], F8, name="sel_t")
                nc.sync.dma_start(
                    out=sel_t[:, :sw], in_=sel[t * 128:(t + 1) * 128, :sw])
                sels[t] = sel_t

            def prefetch(t):
                if t >= NTILES:
                    return
                pre_gather(t)
                pre_sel(t)

            # startup orchestration: gather(0) first, early W chunks, sel(0),
            # then stream the rest so the PE can start by ~9us.
            # PE warm-up: keeps the PE busy from t~0.6us so the p-state
            # ramp completes during the DMA pipeline fill (results unused)
            dumt = wpool.tile([128, 512], BF, name="dumt")
            nc.vector.memset(dumt[:], 0)
            dump = psy.tile([128, D], F32, name="yp", uniquify=False)
            for i in range(20):
                nc.tensor.matmul(out=dump[:], lhsT=dumt[:, :128],
                                 rhs=dumt[:, 256:256 + D], start=True,
                                 stop=True, skip_group_check=True)

            pre_gather(0)
            load_w(0, 3)
            load_w(3, 7)
            pre_sel(0)
            load_w(7, 12)
            pre_gather(1)
            load_w(12, 17)
            pre_sel(1)
            load_w(17, 22)
            prefetch(2)
            load_w(22, 27)
            load_w(27, R1)

            for t in range(NTILES):
                segs, sel_t = ets.pop(t), sels.pop(t)
                outp = pso.tile([128, D], F32)
                ys = [None] * R1
                for r in range(R1 + SKEW):
                    if r < R1:
                        yp = psy.tile([128, D], F32, name="yp")
                        gi = next(i for i, (a, b) in enumerate(GSEG)
                                  if a <= r * P < b)
                        eth, off = segs[gi], r * P - GSEG[gi][0]
                        wtg, rbase = wts[r]
                        if r < FK:
                            lh = eth[:, :, off:off + P, 0:1]
                            ll = eth[:, :, off:off + P, 1:2]
                            wh = wtg[:, r - rbase, 0]
                            wl = wtg[:, r - rbase, 1]
                            nc.tensor.matmul(out=yp[:], lhsT=lh, rhs=wh,
                                             start=True, stop=False,
                                             perf_mode=DR)
                            nc.tensor.matmul(out=yp[:], lhsT=ll, rhs=wh,
                                             start=False, stop=False,
                                             perf_mode=DR)
                            nc.tensor.matmul(out=yp[:], lhsT=lh, rhs=wl,
                                             start=False, stop=True,
                                             perf_mode=DR)
                        else:
                            for c in range(2):
                                a = ((r - rbase) * 2 + c) * D
                                nc.tensor.matmul(
                                    out=yp[:],
                                    lhsT=eth[:, c, off:off + P],
                                    rhs=wtg[:, a:a + D],
                                    start=(c == 0), stop=(c == 1),
                                )
                        ysb = ypool.tile([128, D], BF, name="ysb")
                        if r % 2 == 0:
                            nc.vector.tensor_copy(out=ysb[:], in_=yp[:])
                        else:
                            nc.scalar.copy(out=ysb[:], in_=yp[:])
                        ys[r] = ysb
                    if r >= SKEW:
                        q = r - SKEW
                        ncnt = CAPS[t]
                        nc.tensor.matmul(
                            out=outp[:ncnt, :],
                            lhsT=sel_t[:, q * ncnt:(q + 1) * ncnt],
                            rhs=ys[q][:],
                            start=(q == 0), stop=(q == R1 - 1),
                        )
                prefetch(t + PF)
                ncnt = CAPS[t]
                osb = opool.tile([128, D], BF)
                nc.scalar.activation(out=osb[:ncnt, :], in_=outp[:ncnt, :],
                                     func=Relu, scale=SCALE / WA)
                base = ROW_BASE[t]
                nc.sync.dma_start(
                    out=out[base:base + ncnt, :], in_=osb[:ncnt, :])

    nc.compile()
    return nc


_NC_CACHE = []


def _get_program():
    if not _NC_CACHE:
        _NC_CACHE.append(build_program())
    return _NC_CACHE[0]


# ---------------------------------------------------------------------------
# Entry point
# ---------------------------------------------------------------------------

def kernel(emb_table, weights, neighbors, relations):
    from concourse import bass_utils

    in_maps, perms = prep(emb_table, weights, neighbors, relations)
    nc = _get_program()
    res = bass_utils.run_bass_kernel_spmd(
        nc, in_maps, core_ids=list(range(NCORES)),
    )
    full = np.zeros((N, D), dtype=np.float32)
    for c in range(NCORES):
        full[perms[c]] = res.results[c]["out"].astype(np.float32)
    return full



# revision 63
# speedup vs baseline: 1.1243x; 1.0164x over previous
"""Trainium2 Bass kernel for LoopRelationalGraphConvolution.

Math (matches the jax reference):
    out[n] = relu( SCALE * sum_s  W[rel[n,s]] @ emb[neighbors[n,s]] )
    SCALE  = 1000 / (R1 * S)      (folds the mean over S and the /R1 * 1000)

Design (8 NeuronCores, data-parallel over the 8192-node batch):
  Each core owns 1024 nodes, split into 9 node-tiles ([114]*8 + [112] nodes)
  chosen by a host-side balancer so that every (tile, relation) bucket has
  <=128 edges.  Per tile the device kernel:
    1. dma_gather(transpose=True): fetches the tile's 33*128 edge-slot
       embeddings (bf16, compacted per-core int16 ids) with the embedding dim
       landing on SBUF partitions:  ET[p, c, i] = emb[idx_i][c*128+p].
    2. stage-1 matmuls: per relation-chunk r (128 slots), in 2 K-chunks over D:
       Y[slot, o] += ET[:, c, slots]^T @ W_r[c]       (PSUM, f32)
    3. stage-2 matmul: 0/1 selection matrix reduces edge slots into node rows:
       out_psum[node, o] += SEL_r^T @ Y_bf16          (accumulated over all r)
    4. relu on PSUM->SBUF evacuation, DMA node rows to DRAM.
  The device program is fully static and identical across cores (SPMD); all
  data-dependence lives in the index / selection arrays.  Host post-step
  inverse-permutes rows back to the original node order.
"""

import numpy as np
import ml_dtypes

bf16 = ml_dtypes.bfloat16
fp8 = ml_dtypes.float8_e4m3

# Problem constants (hardcoded per contract).
V = 100000
D = 256
R1 = 33          # relations incl. self-loop
N = 8192
S = 32
NCORES = 8
NPC = N // NCORES          # 1024 nodes per core
NTILES = 9                 # node-tiles per core
CAPS = [114] * 8 + [112]   # nodes per tile (uniform across cores)
ROW_BASE = np.concatenate([[0], np.cumsum(CAPS)]).tolist()
P = 128
NSLOT = R1 * P             # 4224 edge slots per tile
GSPLIT = [0, 2, 6, 12, 22, R1]   # gather segment chunk boundaries
GSEG = [(a * P, b * P) for a, b in zip(GSPLIT, GSPLIT[1:])]
BK = 6      # chunks 0..BK-1 run stage-1 in bf16; chunks BK.. run fp8
            # DoubleRow (two-level fp8, 3 products at 0.5 cyc/row). Slower
            # bf16 chunks go first so early-tile PE consumption is slower
            # and the DMA stays ahead. BK must align with a GSPLIT boundary.
            # chunk c processes relation REL_OF_CHUNK[c] (arbitrary mapping).
WA = 16.0   # pre-scale on W so fp8 residuals stay in e4m3 normal range;
            # folded out in the final relu
REL_OF_CHUNK = np.concatenate([np.arange(R1 - BK, R1), np.arange(R1 - BK)])
CHUNK_OF_REL = np.argsort(REL_OF_CHUNK)
IDXW = NSLOT // 16         # 264 int16 idx columns (16-partition wrap)
UMAX = 32768               # compacted per-core embedding rows (int16 limit)
SCALE = 1000.0 / (R1 * S)

# Software-pipeline skew between stage-1 and stage-2 of consecutive chunks,
# so the PE never stalls on the PSUM->SBUF copy of the current chunk.
SKEW = 6
PF = 3      # tile prefetch depth


# ---------------------------------------------------------------------------
# Host-side preparation
# ---------------------------------------------------------------------------

def _balance_tiles(hist):
    """Assign NPC nodes to NTILES tiles (exactly CAPS[t] nodes each),
    minimizing the max per-(tile, relation) edge count. hist: [NPC, R1].
    Greedy: hardest nodes first, place on the tile minimizing the resulting
    peak bucket."""
    order = np.argsort(-hist.max(axis=1), kind="stable")
    loads = np.zeros((NTILES, R1), dtype=np.int64)
    counts = np.zeros(NTILES, dtype=np.int64)
    tiles = [[] for _ in range(NTILES)]
    for n in order:
        h = hist[n]
        best_t, best_key = -1, None
        for t in range(NTILES):
            if counts[t] >= CAPS[t]:
                continue
            new = loads[t] + h
            key = (int(new.max()), int(loads[t].max()), int(new.sum()))
            if best_key is None or key < best_key:
                best_key, best_t = key, t
        tiles[best_t].append(int(n))
        loads[best_t] += h
        counts[best_t] += 1
    return tiles, loads


def _split2(x):
    """Two-level fp8 decomposition: x ~= hi + lo, both fp8_e4m3."""
    x = np.asarray(x, dtype=np.float32)
    hi = x.astype(fp8)
    lo = (x - hi.astype(np.float32)).astype(fp8)
    return hi, lo


def prep(emb_table, weights, neighbors, relations):
    """Build per-core device arrays. Returns (in_maps, perms)."""
    emb32 = np.asarray(emb_table, dtype=np.float32)
    emb_bf = emb32.astype(bf16)
    w = np.asarray(weights, dtype=np.float32) * WA            # [R1, D_out, D_in]
    # reorder relations into chunk order, bf16 chunks first
    w_rdo = np.ascontiguousarray(
        w.transpose(0, 2, 1)[REL_OF_CHUNK])                   # [chunk, d, o]
    W16_sb = np.ascontiguousarray(
        w_rdo[:BK].reshape(BK, 2, 128, D).transpose(2, 0, 1, 3)
    ).reshape(128, BK * 2 * D).astype(bf16)
    # fp8 part: two-level; W8_sb[p, (((r*2+lvl)*2+c)*D + o)] = W{lvl}[r, c*128+p, o]
    wh, wl = _split2(w_rdo[BK:])
    W8_sb = np.ascontiguousarray(
        np.stack([wh, wl], axis=1).reshape(R1 - BK, 2, 2, 128, D)
        .transpose(3, 0, 1, 2, 4)).reshape(128, (R1 - BK) * 2 * 2 * D)

    neighbors = np.asarray(neighbors).astype(np.int64)
    relations = np.asarray(relations).astype(np.int64)

    in_maps, perms = [], []
    for c in range(NCORES):
        nb = neighbors[c * NPC:(c + 1) * NPC]                 # [NPC, S]
        rel = relations[c * NPC:(c + 1) * NPC]
        uniq, inv = np.unique(nb.ravel(), return_inverse=True)
        inv = inv.reshape(nb.shape).astype(np.int64)
        U = len(uniq)
        assert U <= UMAX, U
        emb_c = np.zeros((UMAX, D), dtype=bf16)
        emb_c[:U] = emb_bf[uniq]
        hi, lo = _split2(emb32[uniq])
        ehl = np.zeros((UMAX, 2 * D), dtype=fp8)              # [hi|lo] pairs
        ehl[:U, 0::2] = hi
        ehl[:U, 1::2] = lo

        hist = np.zeros((NPC, R1), dtype=np.int64)
        np.add.at(hist, (np.repeat(np.arange(NPC), S), rel.ravel()), 1)
        tiles, loads = _balance_tiles(hist)
        assert loads.max() <= P, f"balance failed: max bucket {loads.max()}"

        idx_all = np.zeros((NTILES, 128, IDXW), dtype=np.int16)
        sel_all = np.zeros((NTILES, 128, NSLOT), dtype=fp8)
        perm = []
        for t, nodes in enumerate(tiles):
            nodes = np.array(nodes, dtype=np.int64)
            ncnt = len(nodes)
            assert ncnt == CAPS[t]
            perm.extend((c * NPC + nodes).tolist())
            # edges of this tile
            er = CHUNK_OF_REL[rel[nodes].ravel()]              # chunk per edge
            ei = inv[nodes].ravel()                            # compact nbr id
            ej = np.repeat(np.arange(ncnt), S)                 # local node idx
            order = np.argsort(er, kind="stable")
            er_s, ei_s, ej_s = er[order], ei[order], ej[order]
            # position within relation group
            start = np.searchsorted(er_s, np.arange(R1))
            pos = np.arange(ncnt * S) - start[er_s]
            slot = er_s * P + pos                              # [ncnt*S]
            slots_idx = np.zeros(NSLOT, dtype=np.int16)
            slots_idx[slot] = ei_s
            sel = np.zeros((NSLOT, 128), dtype=fp8)
            sel[slot, ej_s] = 1.0
            # idx wrap per gather segment: idx i at partition i%16, col i//16
            wrapped = np.concatenate(
                [slots_idx[a:b].reshape((b - a) // 16, 16).T
                 for a, b in GSEG], axis=1)                    # [16, IDXW]
            idx_all[t] = np.tile(wrapped, (8, 1))
            # device SEL layout: [part p = slot-in-chunk, free = r*ncnt+node]
            packed = np.ascontiguousarray(
                sel.reshape(R1, P, 128).transpose(1, 0, 2)[:, :, :ncnt]
            ).reshape(P, R1 * ncnt)
            sel_all[t, :, :R1 * ncnt] = packed
        in_maps.append({
            "emb": emb_c,
            "emb8": ehl,
            "wsb16": W16_sb,
            "wsb8": W8_sb,
            "idx": np.ascontiguousarray(idx_all.reshape(NTILES * 128, IDXW)),
            "sel": np.ascontiguousarray(sel_all.reshape(NTILES * 128, NSLOT)),
        })
        perms.append(np.array(perm, dtype=np.int64))

    return in_maps, perms


# ---------------------------------------------------------------------------
# Numpy emulation (bf16-faithful) for validation
# ---------------------------------------------------------------------------

def emulate_core(in_map):
    emb = in_map["emb"]                                        # [UMAX, D] bf16
    xh = in_map["emb8"][:, 0::2].astype(np.float32)
    xl = in_map["emb8"][:, 1::2].astype(np.float32)
    w16 = in_map["wsb16"].reshape(128, BK, 2, D)               # [p, r, c, o]
    w8 = in_map["wsb8"].reshape(128, R1 - BK, 2, 2, D)         # [p, r, lvl, c, o]
    w8f = np.ascontiguousarray(w8.transpose(1, 2, 3, 0, 4)).reshape(
        R1 - BK, 2, 2 * 128, D).astype(np.float32)             # [r, lvl, d, o]
    idx = in_map["idx"].reshape(NTILES, 128, IDXW)
    sel = in_map["sel"].reshape(NTILES, 128, NSLOT)
    out = np.zeros((NPC, D), dtype=np.float32)
    for t in range(NTILES):
        parts, col = [], 0
        for a, b in GSEG:
            w = (b - a) // 16
            parts.append(idx[t, :16, col:col + w].T.reshape(b - a))
            col += w
        slots_idx = np.concatenate(parts)                      # unwrap
        base, ncnt = ROW_BASE[t], CAPS[t]
        out_acc = np.zeros((ncnt, D), dtype=np.float32)
        for r in range(R1):
            sl = slots_idx[r * P:(r + 1) * P]
            if r >= BK:
                rr = r - BK
                Y = (xh[sl] @ w8f[rr, 0] + xl[sl] @ w8f[rr, 0]
                     + xh[sl] @ w8f[rr, 1])
            else:
                Xr = emb[sl].astype(np.float32)                # [128, D]
                Y = (Xr[:, :128] @ w16[:, r, 0, :].astype(np.float32)
                     + Xr[:, 128:] @ w16[:, r, 1, :].astype(np.float32))
            Yb = Y.astype(bf16).astype(np.float32)             # PSUM->SBUF bf16
            selr = sel[t][:, r * ncnt:(r + 1) * ncnt].astype(np.float32)
            out_acc += selr.T @ Yb
        res = np.maximum(out_acc * (SCALE / WA), 0.0)
        out[base:base + ncnt] = res.astype(bf16).astype(np.float32)
    return out


def emulate(emb_table, weights, neighbors, relations):
    in_maps, perms = prep(emb_table, weights, neighbors, relations)
    full = np.zeros((N, D), dtype=np.float32)
    for c in range(NCORES):
        full[perms[c]] = emulate_core(in_maps[c])
    return full


# ---------------------------------------------------------------------------
# Bass program
# ---------------------------------------------------------------------------

def build_program():
    import concourse.bacc as bacc
    import concourse.tile as tile
    import concourse.mybir as mybir

    nc = bacc.Bacc(
        "TRN2", target_bir_lowering=False, debug=False,
        num_devices=NCORES,
    )
    BF = mybir.dt.bfloat16
    F32 = mybir.dt.float32
    I16 = mybir.dt.int16
    F8 = mybir.dt.float8e4
    DR = mybir.MatmulPerfMode.DoubleRow

    emb = nc.dram_tensor("emb", [UMAX, D], BF, kind="ExternalInput").ap()
    emb8 = nc.dram_tensor("emb8", [UMAX, 2 * D], F8,
                          kind="ExternalInput").ap()
    wsb16 = nc.dram_tensor("wsb16", [128, BK * 2 * D], BF,
                           kind="ExternalInput").ap()
    wsb8 = nc.dram_tensor("wsb8", [128, (R1 - BK) * 2 * 2 * D], F8,
                          kind="ExternalInput").ap()
    idx = nc.dram_tensor("idx", [NTILES * 128, IDXW], I16, kind="ExternalInput").ap()
    sel = nc.dram_tensor("sel", [NTILES * 128, NSLOT], F8,
                         kind="ExternalInput").ap()
    out = nc.dram_tensor("out", [NPC, D], BF, kind="ExternalOutput").ap()

    Relu = mybir.ActivationFunctionType.Relu

    with tile.TileContext(nc) as tc:
        with (
            tc.tile_pool(name="wpool", bufs=1) as wpool,
            tc.tile_pool(name="etpool", bufs=PF + 1) as etpool,
            tc.tile_pool(name="selpool", bufs=PF + 1) as selpool,
            tc.tile_pool(name="idxpool", bufs=PF + 1) as idxpool,
            tc.tile_pool(name="ypool", bufs=2 * (SKEW + 1)) as ypool,
            tc.tile_pool(name="opool", bufs=2) as opool,
            tc.tile_pool(name="psy", bufs=SKEW, space="PSUM") as psy,
            tc.tile_pool(name="pso", bufs=2, space="PSUM") as pso,
        ):
            wts = {}

            def load_w(r0, r1):
                # [r0, r1) must lie entirely on one side of BK
                if r0 >= BK:
                    a, b = r0 - BK, r1 - BK
                    wtg = wpool.tile([128, r1 - r0, 2, 2, D], F8,
                                     name=f"wt{r0}", uniquify=False)
                    nc.sync.dma_start(
                        out=wtg[:],
                        in_=wsb8[:, a * 4 * D:b * 4 * D])
                else:
                    wtg = wpool.tile([128, (r1 - r0) * 2 * D], BF,
                                     name=f"wt{r0}", uniquify=False)
                    nc.sync.dma_start(out=wtg[:],
                                      in_=wsb16[:, r0 * 2 * D:r1 * 2 * D])
                for r in range(r0, r1):
                    wts[r] = (wtg, r0)

            ets, sels = {}, {}

            def pre_gather(t):
                idx_t = idxpool.tile([128, IDXW], I16, name="idx_t")
                nc.sync.dma_start(
                    out=idx_t[:], in_=idx[t * 128:(t + 1) * 128, :])
                segs, col = [], 0
                for gi, (a, b) in enumerate(GSEG):
                    n = b - a
                    if a >= BK * P:
                        eth = etpool.tile([128, 4, n], F8, name=f"et{gi}")
                        nc.gpsimd.dma_gather(
                            out_ap=eth[:],
                            in_ap=emb8,
                            idxs_ap=idx_t[:, col:col + n // 16],
                            num_idxs=n,
                            num_idxs_reg=n,
                            elem_size=2 * D,
                            transpose=True,
                            single_packet=False,
                        )
                        # true byte layout: [p][ktile c:2][slot:n][hi/lo b:2]
                        eth = eth[:].rearrange("p f n -> p (f n)").rearrange(
                            "p (c i b) -> p c i b", c=2, i=n, b=2)
                    else:
                        eth = etpool.tile([128, 2, n], BF, name=f"et{gi}")
                        nc.gpsimd.dma_gather(
                            out_ap=eth[:],
                            in_ap=emb,
                            idxs_ap=idx_t[:, col:col + n // 16],
                            num_idxs=n,
                            num_idxs_reg=n,
                            elem_size=D,
                            transpose=True,
                            single_packet=False,
                        )
                    col += n // 16
                    segs.append(eth)
                ets[t] = segs

            def pre_sel(t):
                sw = R1 * CAPS[t]
                sel_t = selpool.tile([128, NSLOT], F8, name="sel_t")
                nc.sync.dma_start(
                    out=sel_t[:, :sw], in_=sel[t * 128:(t + 1) * 128, :sw])
                sels[t] = sel_t

            def prefetch(t):
                if t >= NTILES:
                    return
                pre_gather(t)
                pre_sel(t)

            # startup orchestration: gather(0) first, early W chunks, sel(0),
            # then stream the rest so the PE can start by ~9us.
            # PE warm-up: keeps the PE busy from t~0.6us so the p-state
            # ramp completes during the DMA pipeline fill (results unused)
            dumt = wpool.tile([128, 512], BF, name="dumt")
            nc.vector.memset(dumt[:], 0)
            dump = psy.tile([128, D], F32, name="yp", uniquify=False)
            for i in range(20):
                nc.tensor.matmul(out=dump[:], lhsT=dumt[:, :128],
                                 rhs=dumt[:, 256:256 + D], start=True,
                                 stop=True, skip_group_check=True)

            pre_gather(0)
            load_w(0, 3)
            load_w(3, 6)
            pre_sel(0)
            load_w(6, 12)
            pre_gather(1)
            load_w(12, 17)
            pre_sel(1)
            load_w(17, 22)
            prefetch(2)
            load_w(22, 27)
            load_w(27, R1)

            for t in range(NTILES):
                segs, sel_t = ets.pop(t), sels.pop(t)
                outp = pso.tile([128, D], F32)
                ys = [None] * R1
                for r in range(R1 + SKEW):
                    if r < R1:
                        yp = psy.tile([128, D], F32, name="yp")
                        gi = next(i for i, (a, b) in enumerate(GSEG)
                                  if a <= r * P < b)
                        eth, off = segs[gi], r * P - GSEG[gi][0]
                        wtg, rbase = wts[r]
                        if r >= BK:
                            lh = eth[:, :, off:off + P, 0:1]
                            ll = eth[:, :, off:off + P, 1:2]
                            wh = wtg[:, r - rbase, 0]
                            wl = wtg[:, r - rbase, 1]
                            nc.tensor.matmul(out=yp[:], lhsT=lh, rhs=wh,
                                             start=True, stop=False,
                                             perf_mode=DR)
                            nc.tensor.matmul(out=yp[:], lhsT=ll, rhs=wh,
                                             start=False, stop=False,
                                             perf_mode=DR)
                            nc.tensor.matmul(out=yp[:], lhsT=lh, rhs=wl,
                                             start=False, stop=True,
                                             perf_mode=DR)
                        else:
                            for c in range(2):
                                a = ((r - rbase) * 2 + c) * D
                                nc.tensor.matmul(
                                    out=yp[:],
                                    lhsT=eth[:, c, off:off + P],
                                    rhs=wtg[:, a:a + D],
                                    start=(c == 0), stop=(c == 1),
                                )
                        ysb = ypool.tile([128, D], BF, name="ysb")
                        if r % 2 == 0:
                            nc.vector.tensor_copy(out=ysb[:], in_=yp[:])
                        else:
                            nc.scalar.copy(out=ysb[:], in_=yp[:])
                        ys[r] = ysb
                    if r >= SKEW:
                        q = r - SKEW
                        ncnt = CAPS[t]
                        nc.tensor.matmul(
                            out=outp[:ncnt, :],
                            lhsT=sel_t[:, q * ncnt:(q + 1) * ncnt],
                            rhs=ys[q][:],
                            start=(q == 0), stop=(q == R1 - 1),
                        )
                prefetch(t + PF)
                ncnt = CAPS[t]
                osb = opool.tile([128, D], BF)
                nc.scalar.activation(out=osb[:ncnt, :], in_=outp[:ncnt, :],
                                     func=Relu, scale=SCALE / WA)
                base = ROW_BASE[t]
                nc.sync.dma_start(
                    out=out[base:base + ncnt, :], in_=osb[:ncnt, :])

    nc.compile()
    return nc


_NC_CACHE = []


def _get_program():
    if not _NC_CACHE:
        _NC_CACHE.append(build_program())
    return _NC_CACHE[0]


# ---------------------------------------------------------------------------
# Entry point
# ---------------------------------------------------------------------------

def kernel(emb_table, weights, neighbors, relations):
    from concourse import bass_utils

    in_maps, perms = prep(emb_table, weights, neighbors, relations)
    nc = _get_program()
    res = bass_utils.run_bass_kernel_spmd(
        nc, in_maps, core_ids=list(range(NCORES)),
    )
    full = np.zeros((N, D), dtype=np.float32)
    for c in range(NCORES):
        full[perms[c]] = res.results[c]["out"].astype(np.float32)
    return full



# revision 69
# speedup vs baseline: 1.1260x; 1.0015x over previous
"""Trainium2 Bass kernel for LoopRelationalGraphConvolution.

Math (matches the jax reference):
    out[n] = relu( SCALE * sum_s  W[rel[n,s]] @ emb[neighbors[n,s]] )
    SCALE  = 1000 / (R1 * S)      (folds the mean over S and the /R1 * 1000)

Design (8 NeuronCores, data-parallel over the 8192-node batch):
  Each core owns 1024 nodes, split into 9 node-tiles ([114]*8 + [112] nodes)
  chosen by a host-side balancer so that every (tile, relation) bucket has
  <=128 edges.  Per tile the device kernel:
    1. dma_gather(transpose=True): fetches the tile's 33*128 edge-slot
       embeddings (bf16, compacted per-core int16 ids) with the embedding dim
       landing on SBUF partitions:  ET[p, c, i] = emb[idx_i][c*128+p].
    2. stage-1 matmuls: per relation-chunk r (128 slots), in 2 K-chunks over D:
       Y[slot, o] += ET[:, c, slots]^T @ W_r[c]       (PSUM, f32)
    3. stage-2 matmul: 0/1 selection matrix reduces edge slots into node rows:
       out_psum[node, o] += SEL_r^T @ Y_bf16          (accumulated over all r)
    4. relu on PSUM->SBUF evacuation, DMA node rows to DRAM.
  The device program is fully static and identical across cores (SPMD); all
  data-dependence lives in the index / selection arrays.  Host post-step
  inverse-permutes rows back to the original node order.
"""

import numpy as np
import ml_dtypes

bf16 = ml_dtypes.bfloat16
fp8 = ml_dtypes.float8_e4m3

# Problem constants (hardcoded per contract).
V = 100000
D = 256
R1 = 33          # relations incl. self-loop
N = 8192
S = 32
NCORES = 8
NPC = N // NCORES          # 1024 nodes per core
NTILES = 9                 # node-tiles per core
CAPS = [114] * 8 + [112]   # nodes per tile (uniform across cores)
ROW_BASE = np.concatenate([[0], np.cumsum(CAPS)]).tolist()
P = 128
NSLOT = R1 * P             # 4224 edge slots per tile
GSPLIT = [0, 2, 6, 12, 22, R1]   # gather segment chunk boundaries
GSEG = [(a * P, b * P) for a, b in zip(GSPLIT, GSPLIT[1:])]
BK = 6      # chunks 0..BK-1 run stage-1 in bf16; chunks BK.. run fp8
            # DoubleRow (two-level fp8, 3 products at 0.5 cyc/row). Slower
            # bf16 chunks go first so early-tile PE consumption is slower
            # and the DMA stays ahead. BK must align with a GSPLIT boundary.
            # chunk c processes relation REL_OF_CHUNK[c] (arbitrary mapping).
WA = 16.0   # pre-scale on W so fp8 residuals stay in e4m3 normal range;
            # folded out in the final relu
REL_OF_CHUNK = np.concatenate([np.arange(R1 - BK, R1), np.arange(R1 - BK)])
CHUNK_OF_REL = np.argsort(REL_OF_CHUNK)
IDXW = NSLOT // 16         # 264 int16 idx columns (16-partition wrap)
UMAX = 32768               # compacted per-core embedding rows (int16 limit)
SCALE = 1000.0 / (R1 * S)

# Software-pipeline skew between stage-1 and stage-2 of consecutive chunks,
# so the PE never stalls on the PSUM->SBUF copy of the current chunk.
SKEW = 6
PF = 3      # tile prefetch depth
SELR = 124  # max edges in any (tile, chunk) bucket (balancer-enforced);
            # SEL rows beyond this are all-zero, so stage-2 contracts over
            # SELR slot partitions and the SEL DMA skips the zero rows.


# ---------------------------------------------------------------------------
# Host-side preparation
# ---------------------------------------------------------------------------

def _balance_tiles(hist):
    """Assign NPC nodes to NTILES tiles (exactly CAPS[t] nodes each),
    minimizing the max per-(tile, relation) edge count. hist: [NPC, R1].
    Greedy: hardest nodes first, place on the tile minimizing the resulting
    peak bucket."""
    order = np.argsort(-hist.max(axis=1), kind="stable")
    loads = np.zeros((NTILES, R1), dtype=np.int64)
    counts = np.zeros(NTILES, dtype=np.int64)
    tiles = [[] for _ in range(NTILES)]
    for n in order:
        h = hist[n]
        best_t, best_key = -1, None
        for t in range(NTILES):
            if counts[t] >= CAPS[t]:
                continue
            new = loads[t] + h
            key = (int(new.max()), int(loads[t].max()), int(new.sum()))
            if best_key is None or key < best_key:
                best_key, best_t = key, t
        tiles[best_t].append(int(n))
        loads[best_t] += h
        counts[best_t] += 1
    return tiles, loads


def _split2(x):
    """Two-level fp8 decomposition: x ~= hi + lo, both fp8_e4m3."""
    x = np.asarray(x, dtype=np.float32)
    hi = x.astype(fp8)
    lo = (x - hi.astype(np.float32)).astype(fp8)
    return hi, lo


def prep(emb_table, weights, neighbors, relations):
    """Build per-core device arrays. Returns (in_maps, perms)."""
    emb32 = np.asarray(emb_table, dtype=np.float32)
    emb_bf = emb32.astype(bf16)
    w = np.asarray(weights, dtype=np.float32) * WA            # [R1, D_out, D_in]
    # reorder relations into chunk order, bf16 chunks first
    w_rdo = np.ascontiguousarray(
        w.transpose(0, 2, 1)[REL_OF_CHUNK])                   # [chunk, d, o]
    W16_sb = np.ascontiguousarray(
        w_rdo[:BK].reshape(BK, 2, 128, D).transpose(2, 0, 1, 3)
    ).reshape(128, BK * 2 * D).astype(bf16)
    # fp8 part: two-level; W8_sb[p, (((r*2+lvl)*2+c)*D + o)] = W{lvl}[r, c*128+p, o]
    wh, wl = _split2(w_rdo[BK:])
    W8_sb = np.ascontiguousarray(
        np.stack([wh, wl], axis=1).reshape(R1 - BK, 2, 2, 128, D)
        .transpose(3, 0, 1, 2, 4)).reshape(128, (R1 - BK) * 2 * 2 * D)

    neighbors = np.asarray(neighbors).astype(np.int64)
    relations = np.asarray(relations).astype(np.int64)

    in_maps, perms = [], []
    for c in range(NCORES):
        nb = neighbors[c * NPC:(c + 1) * NPC]                 # [NPC, S]
        rel = relations[c * NPC:(c + 1) * NPC]
        uniq, inv = np.unique(nb.ravel(), return_inverse=True)
        inv = inv.reshape(nb.shape).astype(np.int64)
        U = len(uniq)
        assert U <= UMAX, U
        emb_c = np.zeros((UMAX, D), dtype=bf16)
        emb_c[:U] = emb_bf[uniq]
        hi, lo = _split2(emb32[uniq])
        ehl = np.zeros((UMAX, 2 * D), dtype=fp8)              # [hi|lo] pairs
        ehl[:U, 0::2] = hi
        ehl[:U, 1::2] = lo

        hist = np.zeros((NPC, R1), dtype=np.int64)
        np.add.at(hist, (np.repeat(np.arange(NPC), S), rel.ravel()), 1)
        tiles, loads = _balance_tiles(hist)
        assert loads.max() <= SELR, f"balance failed: max bucket {loads.max()}"

        idx_all = np.zeros((NTILES, 128, IDXW), dtype=np.int16)
        sel_all = np.zeros((NTILES, 128, NSLOT), dtype=fp8)
        perm = []
        for t, nodes in enumerate(tiles):
            nodes = np.array(nodes, dtype=np.int64)
            ncnt = len(nodes)
            assert ncnt == CAPS[t]
            perm.extend((c * NPC + nodes).tolist())
            # edges of this tile
            er = CHUNK_OF_REL[rel[nodes].ravel()]              # chunk per edge
            ei = inv[nodes].ravel()                            # compact nbr id
            ej = np.repeat(np.arange(ncnt), S)                 # local node idx
            order = np.argsort(er, kind="stable")
            er_s, ei_s, ej_s = er[order], ei[order], ej[order]
            # position within relation group
            start = np.searchsorted(er_s, np.arange(R1))
            pos = np.arange(ncnt * S) - start[er_s]
            slot = er_s * P + pos                              # [ncnt*S]
            slots_idx = np.zeros(NSLOT, dtype=np.int16)
            slots_idx[slot] = ei_s
            sel = np.zeros((NSLOT, 128), dtype=fp8)
            sel[slot, ej_s] = 1.0
            # idx wrap per gather segment: idx i at partition i%16, col i//16
            wrapped = np.concatenate(
                [slots_idx[a:b].reshape((b - a) // 16, 16).T
                 for a, b in GSEG], axis=1)                    # [16, IDXW]
            idx_all[t] = np.tile(wrapped, (8, 1))
            # device SEL layout: [part p = slot-in-chunk, free = r*ncnt+node]
            packed = np.ascontiguousarray(
                sel.reshape(R1, P, 128).transpose(1, 0, 2)[:, :, :ncnt]
            ).reshape(P, R1 * ncnt)
            sel_all[t, :, :R1 * ncnt] = packed
        in_maps.append({
            "emb": emb_c,
            "emb8": ehl,
            "wsb16": W16_sb,
            "wsb8": W8_sb,
            "idx": np.ascontiguousarray(idx_all.reshape(NTILES * 128, IDXW)),
            "sel": np.ascontiguousarray(sel_all.reshape(NTILES * 128, NSLOT)),
        })
        perms.append(np.array(perm, dtype=np.int64))

    return in_maps, perms


# ---------------------------------------------------------------------------
# Numpy emulation (bf16-faithful) for validation
# ---------------------------------------------------------------------------

def emulate_core(in_map):
    emb = in_map["emb"]                                        # [UMAX, D] bf16
    xh = in_map["emb8"][:, 0::2].astype(np.float32)
    xl = in_map["emb8"][:, 1::2].astype(np.float32)
    w16 = in_map["wsb16"].reshape(128, BK, 2, D)               # [p, r, c, o]
    w8 = in_map["wsb8"].reshape(128, R1 - BK, 2, 2, D)         # [p, r, lvl, c, o]
    w8f = np.ascontiguousarray(w8.transpose(1, 2, 3, 0, 4)).reshape(
        R1 - BK, 2, 2 * 128, D).astype(np.float32)             # [r, lvl, d, o]
    idx = in_map["idx"].reshape(NTILES, 128, IDXW)
    sel = in_map["sel"].reshape(NTILES, 128, NSLOT)
    out = np.zeros((NPC, D), dtype=np.float32)
    for t in range(NTILES):
        parts, col = [], 0
        for a, b in GSEG:
            w = (b - a) // 16
            parts.append(idx[t, :16, col:col + w].T.reshape(b - a))
            col += w
        slots_idx = np.concatenate(parts)                      # unwrap
        base, ncnt = ROW_BASE[t], CAPS[t]
        out_acc = np.zeros((ncnt, D), dtype=np.float32)
        for r in range(R1):
            sl = slots_idx[r * P:(r + 1) * P]
            if r >= BK:
                rr = r - BK
                Y = (xh[sl] @ w8f[rr, 0] + xl[sl] @ w8f[rr, 0]
                     + xh[sl] @ w8f[rr, 1])
            else:
                Xr = emb[sl].astype(np.float32)                # [128, D]
                Y = (Xr[:, :128] @ w16[:, r, 0, :].astype(np.float32)
                     + Xr[:, 128:] @ w16[:, r, 1, :].astype(np.float32))
            Yb = Y.astype(bf16).astype(np.float32)             # PSUM->SBUF bf16
            selr = sel[t][:, r * ncnt:(r + 1) * ncnt].astype(np.float32)
            out_acc += selr.T @ Yb
        res = np.maximum(out_acc * (SCALE / WA), 0.0)
        out[base:base + ncnt] = res.astype(bf16).astype(np.float32)
    return out


def emulate(emb_table, weights, neighbors, relations):
    in_maps, perms = prep(emb_table, weights, neighbors, relations)
    full = np.zeros((N, D), dtype=np.float32)
    for c in range(NCORES):
        full[perms[c]] = emulate_core(in_maps[c])
    return full


# ---------------------------------------------------------------------------
# Bass program
# ---------------------------------------------------------------------------

def build_program():
    import concourse.bacc as bacc
    import concourse.tile as tile
    import concourse.mybir as mybir

    nc = bacc.Bacc(
        "TRN2", target_bir_lowering=False, debug=False,
        num_devices=NCORES,
    )
    BF = mybir.dt.bfloat16
    F32 = mybir.dt.float32
    I16 = mybir.dt.int16
    F8 = mybir.dt.float8e4
    DR = mybir.MatmulPerfMode.DoubleRow

    emb = nc.dram_tensor("emb", [UMAX, D], BF, kind="ExternalInput").ap()
    emb8 = nc.dram_tensor("emb8", [UMAX, 2 * D], F8,
                          kind="ExternalInput").ap()
    wsb16 = nc.dram_tensor("wsb16", [128, BK * 2 * D], BF,
                           kind="ExternalInput").ap()
    wsb8 = nc.dram_tensor("wsb8", [128, (R1 - BK) * 2 * 2 * D], F8,
                          kind="ExternalInput").ap()
    idx = nc.dram_tensor("idx", [NTILES * 128, IDXW], I16, kind="ExternalInput").ap()
    sel = nc.dram_tensor("sel", [NTILES * 128, NSLOT], F8,
                         kind="ExternalInput").ap()
    out = nc.dram_tensor("out", [NPC, D], BF, kind="ExternalOutput").ap()

    Relu = mybir.ActivationFunctionType.Relu

    with tile.TileContext(nc) as tc:
        with (
            tc.tile_pool(name="wpool", bufs=1) as wpool,
            tc.tile_pool(name="etpool", bufs=PF + 1) as etpool,
            tc.tile_pool(name="selpool", bufs=PF + 1) as selpool,
            tc.tile_pool(name="idxpool", bufs=PF + 1) as idxpool,
            tc.tile_pool(name="ypool", bufs=2 * (SKEW + 1)) as ypool,
            tc.tile_pool(name="opool", bufs=2) as opool,
            tc.tile_pool(name="psy", bufs=SKEW, space="PSUM") as psy,
            tc.tile_pool(name="pso", bufs=2, space="PSUM") as pso,
        ):
            wts = {}

            def load_w(r0, r1):
                # [r0, r1) must lie entirely on one side of BK
                if r0 >= BK:
                    a, b = r0 - BK, r1 - BK
                    wtg = wpool.tile([128, r1 - r0, 2, 2, D], F8,
                                     name=f"wt{r0}", uniquify=False)
                    nc.sync.dma_start(
                        out=wtg[:],
                        in_=wsb8[:, a * 4 * D:b * 4 * D])
                else:
                    wtg = wpool.tile([128, (r1 - r0) * 2 * D], BF,
                                     name=f"wt{r0}", uniquify=False)
                    nc.sync.dma_start(out=wtg[:],
                                      in_=wsb16[:, r0 * 2 * D:r1 * 2 * D])
                for r in range(r0, r1):
                    wts[r] = (wtg, r0)

            ets, sels = {}, {}

            def pre_gather(t):
                idx_t = idxpool.tile([128, IDXW], I16, name="idx_t")
                nc.sync.dma_start(
                    out=idx_t[:], in_=idx[t * 128:(t + 1) * 128, :])
                segs, col = [], 0
                for gi, (a, b) in enumerate(GSEG):
                    n = b - a
                    if a >= BK * P:
                        eth = etpool.tile([128, 4, n], F8, name=f"et{gi}")
                        nc.gpsimd.dma_gather(
                            out_ap=eth[:],
                            in_ap=emb8,
                            idxs_ap=idx_t[:, col:col + n // 16],
                            num_idxs=n,
                            num_idxs_reg=n,
                            elem_size=2 * D,
                            transpose=True,
                            single_packet=False,
                        )
                        # true byte layout: [p][ktile c:2][slot:n][hi/lo b:2]
                        eth = eth[:].rearrange("p f n -> p (f n)").rearrange(
                            "p (c i b) -> p c i b", c=2, i=n, b=2)
                    else:
                        eth = etpool.tile([128, 2, n], BF, name=f"et{gi}")
                        nc.gpsimd.dma_gather(
                            out_ap=eth[:],
                            in_ap=emb,
                            idxs_ap=idx_t[:, col:col + n // 16],
                            num_idxs=n,
                            num_idxs_reg=n,
                            elem_size=D,
                            transpose=True,
                            single_packet=False,
                        )
                    col += n // 16
                    segs.append(eth)
                ets[t] = segs

            def pre_sel(t):
                sw = R1 * CAPS[t]
                sel_t = selpool.tile([SELR, NSLOT], F8, name="sel_t")
                nc.sync.dma_start(
                    out=sel_t[:, :sw],
                    in_=sel[t * 128:t * 128 + SELR, :sw])
                sels[t] = sel_t

            def prefetch(t):
                if t >= NTILES:
                    return
                pre_gather(t)
                pre_sel(t)

            # startup orchestration: gather(0) first, early W chunks, sel(0),
            # then stream the rest so the PE can start by ~9us.
            # PE warm-up: keeps the PE busy from t~0.6us so the p-state
            # ramp completes during the DMA pipeline fill (results unused)
            dumt = wpool.tile([128, 512], BF, name="dumt")
            nc.vector.memset(dumt[:], 0)
            dump = psy.tile([128, D], F32, name="yp", uniquify=False)
            for i in range(20):
                nc.tensor.matmul(out=dump[:], lhsT=dumt[:, :128],
                                 rhs=dumt[:, 256:256 + D], start=True,
                                 stop=True, skip_group_check=True)

            pre_gather(0)
            load_w(0, 3)
            load_w(3, 6)
            pre_sel(0)
            load_w(6, 12)
            pre_gather(1)
            load_w(12, 17)
            pre_sel(1)
            load_w(17, 22)
            prefetch(2)
            load_w(22, 27)
            load_w(27, R1)

            for t in range(NTILES):
                segs, sel_t = ets.pop(t), sels.pop(t)
                outp = pso.tile([128, D], F32)
                ys = [None] * R1
                for r in range(R1 + SKEW):
                    if r < R1:
                        yp = psy.tile([128, D], F32, name="yp")
                        gi = next(i for i, (a, b) in enumerate(GSEG)
                                  if a <= r * P < b)
                        eth, off = segs[gi], r * P - GSEG[gi][0]
                        wtg, rbase = wts[r]
                        if r >= BK:
                            lh = eth[:, :, off:off + P, 0:1]
                            ll = eth[:, :, off:off + P, 1:2]
                            wh = wtg[:, r - rbase, 0]
                            wl = wtg[:, r - rbase, 1]
                            nc.tensor.matmul(out=yp[:], lhsT=lh, rhs=wh,
                                             start=True, stop=False,
                                             perf_mode=DR)
                            nc.tensor.matmul(out=yp[:], lhsT=ll, rhs=wh,
                                             start=False, stop=False,
                                             perf_mode=DR)
                            nc.tensor.matmul(out=yp[:], lhsT=lh, rhs=wl,
                                             start=False, stop=True,
                                             perf_mode=DR)
                        else:
                            for c in range(2):
                                a = ((r - rbase) * 2 + c) * D
                                nc.tensor.matmul(
                                    out=yp[:],
                                    lhsT=eth[:, c, off:off + P],
                                    rhs=wtg[:, a:a + D],
                                    start=(c == 0), stop=(c == 1),
                                )
                        ysb = ypool.tile([128, D], BF, name="ysb")
                        if r % 2 == 0:
                            nc.vector.tensor_copy(out=ysb[:], in_=yp[:])
                        else:
                            nc.scalar.copy(out=ysb[:], in_=yp[:])
                        ys[r] = ysb
                    if r >= SKEW:
                        q = r - SKEW
                        ncnt = CAPS[t]
                        nc.tensor.matmul(
                            out=outp[:ncnt, :],
                            lhsT=sel_t[:, q * ncnt:(q + 1) * ncnt],
                            rhs=ys[q][:SELR, :],
                            start=(q == 0), stop=(q == R1 - 1),
                        )
                prefetch(t + PF)
                ncnt = CAPS[t]
                osb = opool.tile([128, D], BF)
                nc.scalar.activation(out=osb[:ncnt, :], in_=outp[:ncnt, :],
                                     func=Relu, scale=SCALE / WA)
                base = ROW_BASE[t]
                nc.sync.dma_start(
                    out=out[base:base + ncnt, :], in_=osb[:ncnt, :])

    nc.compile()
    return nc


_NC_CACHE = []


def _get_program():
    if not _NC_CACHE:
        _NC_CACHE.append(build_program())
    return _NC_CACHE[0]


# ---------------------------------------------------------------------------
# Entry point
# ---------------------------------------------------------------------------

def kernel(emb_table, weights, neighbors, relations):
    from concourse import bass_utils

    in_maps, perms = prep(emb_table, weights, neighbors, relations)
    nc = _get_program()
    res = bass_utils.run_bass_kernel_spmd(
        nc, in_maps, core_ids=list(range(NCORES)),
    )
    full = np.zeros((N, D), dtype=np.float32)
    for c in range(NCORES):
        full[perms[c]] = res.results[c]["out"].astype(np.float32)
    return full

